# revision 1
# baseline (speedup 1.0000x reference)
"""DDSP core synthesizer kernel for Trainium2 (8 NeuronCores, data-parallel).

Reference computation (per row of B=32, T=64000):
    f0_hz = 20*exp(f0); phase = cumsum(2*pi*f0_hz/SR)
    hw    = sum_k sin(phase*k)/k   (k = 1..60)
    audio = mix*hw*loud + (1-mix)*noise*loud;  out = audio / (max|audio| + 1e-6)

Device algorithm (everything in "turns" = phase/2pi), natural layout
[128 partitions = 4 rows x 32 blocks, 2000 free = time-in-block]:
    inc  = exp(f0 + ln(20/SR))                    [ACT Exp]
    u    = blocked cumsum + triangular-matmul offs [DVE scan + PE]
    u1   = frac(u)  in [-0.5, 0.5]                 [custom DVE FRAC_AFFINE]
    per harmonic k = 1..60:
        v_k = frac(k*u1)                           [custom DVE FRAC_AFFINE, 1 op]
        s_k = sin(2pi*v_k) -> bf16                 [ACT Sin, scale=2pi]
        hw += diag(1/k) @ s_k                      [PE accumulate in PSUM]
    epilogue: audio = A*hw + B with A = loud*mix, B = loud*noise*(1-mix)
              (A, B precomputed on GpSimd during the k-loop);
              peak-normalize per row (free-reduce + 32x32 transpose trick).

The custom DVE op (v = x - ((x+M)-M), x = Src0*C0 + C1, M = magic rint
constant) replaces the baseline's two extra PE passes (f16 x-gen matmul +
negi-subtract matmul) and the PSUM-sourced DVE rint, cutting PE work per
pass from 3 matmul sweeps to 1 and DVE work to a single SBUF-sourced op.

Sharding: pure data parallel, 4 rows per core, SPMD on cores 0-7.
"""

import sys

sys.path.insert(0, "/opt/trn_rl_repo")

import numpy as np
import ml_dtypes
from contextlib import ExitStack

import concourse.bass as bass
import concourse.tile as tile
from concourse import bacc, mybir
from concourse import bass_utils

f32 = np.float32
dt = mybir.dt

SR = 44100.0
H = 60                      # harmonics
B, T = 32, 64000
NCORES = 8
RPC = B // NCORES           # rows per core = 4
P = 128                     # SBUF partitions
FD = T * RPC // P           # free dim of master tiles = 2000
BPR = P // RPC              # blocks per row = 32
PI = float(np.pi)
MAGIC = float(1.5 * 2.0 ** 23)
EXP_BIAS = float(np.log(20.0 / SR))

_cache = {}


def _register_frac_op():
    """Register FRAC_AFFINE_ANT: out = x - ((x + M) - M), x = Src0*C0 + C1.

    C0/C1 are per-partition scalars (or literals), M = imm2 (the fp32 magic
    rint constant). 5 ALU stages, well under the 8-stage DVE budget."""
    if "op" in _cache.get("frac", {}):
        return _cache["frac"]["op"]
    from concourse import dve_ops
    from concourse.dve_spec import Spec, Src0, C0, C1, C2, lower
    from concourse.dve_spec import _has_src1 as has_src1
    from concourse.dve_uop import DveOpSpec
    from concourse.dve_table_gen import dve_ver_for

    name = "FRAC_AFFINE_ANT"

    def ref(in0, in1, s0, s1, imm2):
        x = (in0.astype(f32) * f32(s0) + f32(s1)).astype(f32)
        t = (x + f32(imm2)).astype(f32)
        w = (t - f32(imm2)).astype(f32)
        return (x - w).astype(f32)

    x = Src0 * C0 + C1
    spec = Spec(body=x - ((x + C2) - C2), reference=ref)

    if name not in dve_ops._SUB_OPCODE_FOR_NAME:
        row = max(dve_ops._SUB_OPCODE_FOR_NAME.values()) + 1
        assert row < 0x20
        dve_ops._SUB_OPCODE_FOR_NAME[name] = row

    ver = dve_ver_for("TRN2")
    tmp = DveOpSpec(
        name=name,
        opcode=dve_ops.get_dve_sub_opcode(name),
        uops=lower(spec, ver=ver),
        rd1_en=has_src1(spec),
    )
    op = dve_ops.DveOp(name, spec, subdim=False, uops_sha={ver: tmp.sha(ver)})
    if not any(o.name == name for o in dve_ops.OPS):
        dve_ops.OPS.append(op)
    dve_ops.CUSTOM_DVE_SPECS[name] = spec
    _cache["frac"] = {"op": op}
    return op


def _consts():
    # lt: exclusive-prefix matmul weights. offs[m] = sum_k lt[k, m] * totals[k]
    kk, mm_ = np.meshgrid(np.arange(P), np.arange(P), indexing="ij")
    lt = ((kk // BPR == mm_ // BPR) & (kk % BPR < mm_ % BPR)).astype(f32)

    # diags[k] = diag(1/(k+1)) as 128x128 lhsT for the PSUM accumulate
    diags = np.zeros((H, P, P), dtype=np.float64)
    for k in range(H):
        np.fill_diagonal(diags[k], 1.0 / (k + 1))
    diags = diags.astype(ml_dtypes.bfloat16)
    return {"lt": lt, "diags": diags}


def _build():
    frac_op = _register_frac_op()
    nc = bacc.Bacc("TRN2", target_bir_lowering=False, debug=False,
                   enable_asserts=True, num_devices=NCORES)

    f0_d = nc.dram_tensor("f0", [P, FD], dt.float32, kind="ExternalInput")
    loud_d = nc.dram_tensor("loud", [P, FD], dt.float32, kind="ExternalInput")
    mix_d = nc.dram_tensor("mix", [P, FD], dt.float32, kind="ExternalInput")
    noise_d = nc.dram_tensor("noise", [P, FD], dt.float32, kind="ExternalInput")
    lt_d = nc.dram_tensor("lt", [P, P], dt.float32, kind="ExternalInput")
    diags_d = nc.dram_tensor("diags", [H, P, P], dt.bfloat16, kind="ExternalInput")
    out_d = nc.dram_tensor("audio", [P, FD], dt.float32, kind="ExternalOutput")

    AF = mybir.ActivationFunctionType
    ALU = mybir.AluOpType

    with tile.TileContext(nc) as tc, ExitStack() as ctx:
        pool = ctx.enter_context(tc.tile_pool(name="sb", bufs=1))
        vpool = ctx.enter_context(tc.tile_pool(name="vp", bufs=3))
        spool = ctx.enter_context(tc.tile_pool(name="sp", bufs=3))
        hpool = ctx.enter_context(tc.tile_pool(name="hps", bufs=1, space="PSUM"))
        opool = ctx.enter_context(tc.tile_pool(name="ops", bufs=1, space="PSUM"))

        exp_bias = pool.tile([P, 1], dt.float32, tag="cbias_exp")
        nc.vector.memset(exp_bias[:], EXP_BIAS)
        zero_bias = pool.tile([P, 1], dt.float32, tag="cbias_zero")
        nc.vector.memset(zero_bias[:], 0.0)

        # ---- input DMA ----
        f0 = pool.tile([P, FD], dt.float32, tag="scr", bufs=4, name="f0")
        nc.sync.dma_start(f0[:], f0_d.ap())
        lt = pool.tile([P, P], dt.float32)
        nc.gpsimd.dma_start(lt[:], lt_d.ap())
        diags = pool.tile([P, H, P], dt.bfloat16)
        nc.gpsimd.dma_start(diags[:], diags_d.ap().rearrange("k p m -> p k m"))
        loud = pool.tile([P, FD], dt.float32, tag="loud")
        nc.scalar.dma_start(loud[:], loud_d.ap())
        mix = pool.tile([P, FD], dt.float32, tag="mix")
        nc.scalar.dma_start(mix[:], mix_d.ap())
        noise = pool.tile([P, FD], dt.float32, tag="noise")
        nc.scalar.dma_start(noise[:], noise_d.ap())

        # ---- stage 1: phase accumulation (turns) ----
        inc = pool.tile([P, FD], dt.float32, tag="scr", bufs=4, name="inc")
        nc.scalar.activation(inc[:], f0[:], AF.Exp, bias=exp_bias[:, 0:1], scale=1.0)

        local = pool.tile([P, FD], dt.float32, tag="scr", bufs=4, name="local")
        nc.vector.tensor_tensor_scan(local[:], inc[:], inc[:], 0.0,
                                     ALU.add, ALU.bypass)

        offs_ps = opool.tile([P, 1], dt.float32, tag="offs")
        nc.tensor.matmul(offs_ps[:], lt[:], local[:, FD - 1:FD],
                         start=True, stop=True)
        offs = pool.tile([P, 1], dt.float32)
        nc.vector.tensor_copy(offs[:], offs_ps[:])

        # u1 = frac(local + offs) in one custom-DVE op
        u1 = pool.tile([P, FD], dt.float32, tag="u1")
        nc.vector._custom_dve(frac_op, out=u1[:], in0=local[:],
                              s0=1.0, s1=offs[:, 0:1], imm2=MAGIC)

        # ---- epilogue prework on GpSimd (overlaps the k-loop) ----
        # audio = A*hw + B;  A = loud*mix, B = loud*noise*(1-mix)
        A = pool.tile([P, FD], dt.float32, tag="A")
        nc.gpsimd.tensor_tensor(A[:], loud[:], mix[:], ALU.mult)
        ln_ = pool.tile([P, FD], dt.float32, tag="ln")
        nc.gpsimd.tensor_tensor(ln_[:], loud[:], noise[:], ALU.mult)
        lnm = pool.tile([P, FD], dt.float32, tag="lnm")
        nc.gpsimd.tensor_tensor(lnm[:], ln_[:], mix[:], ALU.mult)
        Bt = pool.tile([P, FD], dt.float32, tag="Bt")
        nc.gpsimd.tensor_tensor(Bt[:], ln_[:], lnm[:], ALU.subtract)

        # ---- k-loop: v_k = frac(k*u1); s_k = sin(2pi v_k); hw += s_k/k ----
        hw = hpool.tile([P, 4, 512], dt.float32, tag="hw")
        for k in range(1, H + 1):
            v = vpool.tile([P, FD], dt.float32, tag="v")
            nc.vector._custom_dve(frac_op, out=v[:], in0=u1[:],
                                  s0=float(k), s1=0.0, imm2=MAGIC)
            s = spool.tile([P, FD], dt.bfloat16, tag="s")
            nc.scalar.activation(s[:], v[:], AF.Sin,
                                 bias=zero_bias[:, 0:1], scale=2.0 * PI)
            for qo in range(0, FD, 512):
                qn = min(512, FD - qo)
                nc.tensor.matmul(hw[:, qo // 512, 0:qn],
                                 diags[:, k - 1, :], s[:, qo:qo + qn],
                                 start=(k == 1), stop=(k == H))

        # ---- epilogue: audio = A*hw + B, then per-row peak normalize ----
        hw_flat = hw[:].rearrange("p q f -> p (q f)")[:, 0:FD]
        e1 = pool.tile([P, FD], dt.float32, tag="e1")
        nc.vector.tensor_tensor(e1[:], A[:], hw_flat, ALU.mult)
        audio = pool.tile([P, FD], dt.float32, tag="audio")
        nc.vector.tensor_tensor(audio[:], e1[:], Bt[:], ALU.add)

        # per-row peak: free-dim abs-max then 32x32 block transpose trick
        pk = pool.tile([P, 1], dt.float32, tag="pk")
        nc.vector.tensor_reduce(pk[:], audio[:], axis=mybir.AxisListType.X,
                                op=ALU.max, apply_absolute_value=True)
        pkr = pool.tile([P, 32], dt.float32, tag="pkr")
        nc.vector.tensor_copy(pkr[:], pk[:, 0:1].to_broadcast((P, 32)))
        pkt = pool.tile([P, 32], dt.float32, tag="pkt")
        nc.vector.transpose(pkt[:], pkr[:])
        rowmax = pool.tile([P, 1], dt.float32, tag="rowmax")
        nc.vector.tensor_reduce(rowmax[:], pkt[:],
                                axis=mybir.AxisListType.X, op=ALU.max)
        pke = pool.tile([P, 1], dt.float32, tag="pke")
        nc.vector.tensor_scalar(pke[:], rowmax[:], 1e-6, None, ALU.add)
        rcp = pool.tile([P, 1], dt.float32, tag="rcp")
        nc.vector.reciprocal(rcp[:], pke[:])
        outt = pool.tile([P, FD], dt.float32, tag="outt")
        nc.vector.tensor_scalar(outt[:], audio[:], rcp[:, 0:1], None, ALU.mult)
        nc.sync.dma_start(out_d.ap(), outt[:])

    nc.compile()
    return nc


def kernel(f0, loudness, harmonic_mix, noise):
    if "nc" not in _cache:
        _cache["nc"] = _build()
        _cache["consts"] = _consts()
    nc = _cache["nc"]
    consts = _cache["consts"]

    def shard(a, c):
        return np.ascontiguousarray(
            a[c * RPC:(c + 1) * RPC].astype(f32, copy=False).reshape(P, FD))

    in_maps = []
    for c in range(NCORES):
        in_maps.append({
            "f0": shard(f0, c),
            "loud": shard(loudness, c),
            "mix": shard(harmonic_mix, c),
            "noise": shard(noise, c),
            **consts,
        })

    res = bass_utils.run_bass_kernel_spmd(nc, in_maps, core_ids=list(range(NCORES)))
    outs = [res.results[c]["audio"].reshape(RPC, T) for c in range(NCORES)]
    return np.concatenate(outs, axis=0)



# revision 2
# speedup vs baseline: 1.0572x; 1.0572x over previous
"""DDSP core synthesizer kernel for Trainium2 (8 NeuronCores, data-parallel).

Reference computation (per row of B=32, T=64000):
    f0_hz = 20*exp(f0); phase = cumsum(2*pi*f0_hz/SR)
    hw    = sum_k sin(phase*k)/k   (k = 1..60)
    audio = mix*hw*loud + (1-mix)*noise*loud;  out = audio / (max|audio| + 1e-6)

Device algorithm (phase kept in "turns"), layout [128 partitions = 4 rows x
32 blocks, 2000 free = time-in-block]:
    inc  = exp(f0 + ln(20/SR))                     [ACT Exp]
    u    = single-src cumsum (custom DVE scan) + triangular-matmul offs [PE]
    u1   = frac(u) in [-0.5, 0.5]                  [custom DVE, 1x]
    per harmonic k, grouped in quads q = [4g+1..4g+4]:
        v_k = frac(k*u1)     [custom DVE FRAC4X - hand-authored 2x_2p uop:
                              4-slice chain duplicated across the 8-slice
                              pipe, both SBUF read ports, ~1.44us/op]
        k <= M_ACT: s_k = sin(2pi v)/1 -> bf16     [one ACT Sin per quad]
        k >  M_ACT: s_k = P7(v_k) -> bf16          [custom DVE SINP7, deg-7
                              odd minimax poly, leading coeff normalized to
                              -1 so constants are k-independent; |a7|/k goes
                              into the PE diag]
        hw += diag(w_k) @ s_k  in PSUM             [PE; w_k = 1/k or |a7|/k]
    epilogue: audio = A*hw + B (A = loud*mix, B = loud*noise*(1-mix),
              precomputed on GpSimd during the k-loop); per-row peak
              normalize (free-reduce + 32x32 transpose trick).

Sharding: pure data parallel, 4 rows per core, SPMD on cores 0-7.
"""

import sys

sys.path.insert(0, "/opt/trn_rl_repo")

import numpy as np
import ml_dtypes
from contextlib import ExitStack

import concourse.bass as bass
import concourse.tile as tile
from concourse import bacc, mybir
from concourse import bass_utils

f32 = np.float32
dt = mybir.dt

SR = 44100.0
H = 60                      # harmonics
B, T = 32, 64000
NCORES = 8
RPC = B // NCORES           # rows per core = 4
P = 128                     # SBUF partitions
FD = T * RPC // P           # free dim of master tiles = 2000
BPR = P // RPC              # blocks per row = 32
PI = float(np.pi)
MAGIC = float(1.5 * 2.0 ** 23)
EXP_BIAS = float(np.log(20.0 / SR))

# deg-7 odd minimax fit of sin(2*pi*f) on f in [-0.5, 0.5]; max err 2.5e-4
A1, A3, A5, A7 = 6.27863591, -41.09374848, 77.93051701, 56.08683302
B1, B3, B5 = A1 / A7, A3 / A7, A5 / A7     # normalized: P = f(B1+y(B3+y(B5-y)))

M_ACT = 53                  # harmonics 1..M_ACT on ACT Sin; rest on DVE poly
QUAD = 4                    # harmonics per v/s tile group

_cache = {}


def _register_ops():
    """Custom DVE ops. FRAC4X additionally carries a hand-written 2x_2p
    (two-read-port) uop program + perf_max=2 so the engine runs it at
    2 elem/cycle/lane for fp32 SBUF single-source calls."""
    if "ops" in _cache:
        return _cache["ops"]
    from concourse import dve_ops
    from concourse.dve_spec import (Spec, Src0, C0, C1, C2, lower, scan, sq,
                                    AluOp)
    from concourse.dve_spec import _has_src1 as has_src1
    from concourse.dve_uop import (DveOpSpec, UopConfig, InpSel, AluInp,
                                   OutSel, OutPath, Trigger, ENABLE, DelayInp)
    from concourse.dve_uop import AluOp as UAluOp
    from concourse.dve_table_gen import dve_ver_for

    ver = dve_ver_for("TRN2")
    ops = {}

    def base_reg(name, spec, myspec=None):
        if name not in dve_ops._SUB_OPCODE_FOR_NAME:
            row = max(dve_ops._SUB_OPCODE_FOR_NAME.values()) + 1
            assert row < 0x20
            dve_ops._SUB_OPCODE_FOR_NAME[name] = row
        if myspec is None:
            myspec_ = DveOpSpec(
                name=name, opcode=dve_ops.get_dve_sub_opcode(name),
                uops=lower(spec, ver=ver), rd1_en=has_src1(spec))
        else:
            myspec_ = myspec
            myspec_.opcode = dve_ops.get_dve_sub_opcode(name)
            myspec_.validate(ver)

        class _Op(dve_ops.DveOp):
            def compile(self, ver_):
                return myspec_

        op = _Op(name, spec, subdim=False, uops_sha={ver: myspec_.sha(ver)})
        if not any(o.name == name for o in dve_ops.OPS):
            dve_ops.OPS.append(op)
        dve_ops.CUSTOM_DVE_SPECS[name] = spec
        ops[name] = op
        return op

    # ---- FRAC_AFFINE (1x, 5 slices): f = x - rint(x), x = Src0*C0 + C1 ----
    def fa_ref(in0, in1, s0, s1, imm2):
        x = (in0.astype(f32) * f32(s0) + f32(s1)).astype(f32)
        w = ((x + f32(imm2)).astype(f32) - f32(imm2)).astype(f32)
        return (x - w).astype(f32)
    xa = Src0 * C0 + C1
    base_reg("FRAC_AFF_ANT", Spec(body=xa - ((xa + C2) - C2), reference=fa_ref))

    # ---- CUMSUM1 (1x, single-source scan) ----
    def cs_ref(in0, in1, s0, s1, imm2):
        return np.cumsum(in0.astype(f32), axis=-1, dtype=f32)
    base_reg("CUMSUM1_ANT", Spec(body=scan(AluOp.ADD, Src0), reference=cs_ref))

    # ---- SINP7 (1x, 7 slices): out = Src0*(C0 + y*(C1 + y*(C2 - y))) ----
    def sp_ref(in0, in1, s0, s1, imm2):
        f = in0.astype(f32)
        y = (f * f).astype(f32)
        t = (f32(imm2) - y).astype(f32)
        t = (y * t).astype(f32)
        t = (t + f32(s1)).astype(f32)
        t = (y * t).astype(f32)
        t = (t + f32(s0)).astype(f32)
        return (f * t).astype(f32)
    yq = sq(Src0)
    base_reg("SINP7_ANT", Spec(body=Src0 * (C0 + yq * (C1 + yq * (C2 - yq))),
                               reference=sp_ref))

    # ---- FRAC4X (2x_2p): f = x - rint(x), x = Src0*C0 ----
    def f4_ref(in0, in1, s0, s1, imm2):
        x = (in0.astype(f32) * f32(s0)).astype(f32)
        w = ((x + f32(imm2)).astype(f32) - f32(imm2)).astype(f32)
        return (x - w).astype(f32)
    xb = Src0 * C0
    spec4 = Spec(body=xb - ((xb + C2) - C2), reference=f4_ref)
    uops_1x = lower(spec4, ver=ver)

    u = UopConfig()
    u.enable_input(InpSel.SRC_0, 1)    # chain 0
    u.enable_input(InpSel.CONST_0, 2)  # chain 1 (k)
    u.enable_input(InpSel.CONST_2, 3)  # chain 2 (magic)
    u.enable_input(InpSel.SRC_1, 4)    # chain 3 (element i+1 via port 1)
    u.require_inp0 = ENABLE
    u.require_inp1 = ENABLE
    u.trigger = (Trigger.SRC_TENSOR_DONE, Trigger.NONE, Trigger.NONE)
    b = u.datapath_config
    PD, PA = DelayInp.PREV_DELAY, DelayInp.PREV_ALU_OUT
    # chain A (elem i): blocks 0-3; chain B (elem i+1): blocks 4-7
    b[0].enable_alu(UAluOp.MULTIPLY, AluInp.PREV_DELAY_0, AluInp.PREV_DELAY_1)
    for c in (0, 1, 2, 3):
        b[0].enable_delay_from_src(PD, c)
    b[1].enable_alu(UAluOp.ADD, AluInp.PREV_ALU_OUT, AluInp.PREV_DELAY_2)
    b[1].enable_delay_from_src(PA, 0)
    for c in (1, 2, 3):
        b[1].enable_delay_from_src(PD, c)
    b[2].enable_alu(UAluOp.SUBTRACT, AluInp.PREV_ALU_OUT, AluInp.PREV_DELAY_2)
    for c in (0, 1, 2, 3):
        b[2].enable_delay_from_src(PD, c)
    b[3].enable_alu(UAluOp.SUBTRACT, AluInp.PREV_DELAY_0, AluInp.PREV_ALU_OUT)
    for c in (1, 2, 3):
        b[3].enable_delay_from_src(PD, c)
    b[4].enable_alu(UAluOp.MULTIPLY, AluInp.PREV_DELAY_3, AluInp.PREV_DELAY_1)
    b[4].enable_delay_from_src(PA, 0)
    b[4].enable_delay_from_src(PD, 2)
    b[5].enable_alu(UAluOp.ADD, AluInp.PREV_ALU_OUT, AluInp.PREV_DELAY_2)
    b[5].enable_delay_from_src(PA, 3)
    for c in (0, 2):
        b[5].enable_delay_from_src(PD, c)
    b[6].enable_alu(UAluOp.SUBTRACT, AluInp.PREV_ALU_OUT, AluInp.PREV_DELAY_2)
    for c in (0, 3):
        b[6].enable_delay_from_src(PD, c)
    b[7].enable_alu(UAluOp.SUBTRACT, AluInp.PREV_DELAY_3, AluInp.PREV_ALU_OUT)
    b[7].enable_delay_from_src(PD, 0)
    u.enable_output(OutSel.DELAY_0, OutPath.WR0_LO)   # f_A
    u.enable_output(OutSel.ALU_OUT, OutPath.WR1_LO)   # f_B

    spec4x = DveOpSpec(
        name="FRAC4X_ANT", uops=uops_1x, uops_2x=[uops_1x[0]],
        uops_2x_2p=[u], uops_4x=None, perf_max=2, rd1_en=has_src1(spec4))
    base_reg("FRAC4X_ANT", spec4, myspec=spec4x)

    _patch_perf_max({"FRAC4X_ANT": 2})
    _cache["ops"] = ops
    return ops


def _patch_perf_max(pm_ops):
    """_custom_dve builds InstCustomDveAnt without perf_max (byte-36[7:6]) and
    add_instruction stores a copy, so swap the constructor symbol for a
    factory that injects it for our 2x-capable ops."""
    from concourse import bass_isa
    real = mybir.InstCustomDveAnt
    if getattr(bass_isa.InstCustomDveAnt, "_pm_patched", False):
        return

    def make(*args, **kw):
        pm = pm_ops.get(kw.get("op_name"))
        if pm is not None:
            kw.setdefault("perf_max", pm)
        return real(*args, **kw)

    make._pm_patched = True
    bass_isa.InstCustomDveAnt = make


def _consts():
    # lt: exclusive-prefix matmul weights. offs[m] = sum_k lt[k, m] * totals[k]
    kk, mm_ = np.meshgrid(np.arange(P), np.arange(P), indexing="ij")
    lt = ((kk // BPR == mm_ // BPR) & (kk % BPR < mm_ % BPR)).astype(f32)

    # diags[k] = diag(w_k) as 128x128 lhsT for the PSUM accumulate:
    # w_k = 1/k for ACT harmonics (s = sin), |a7|/k for DVE harmonics
    # (s = sin/|a7| from the normalized poly).
    diags = np.zeros((H, P, P), dtype=np.float64)
    for k in range(1, H + 1):
        w = (1.0 / k) if k <= M_ACT else (A7 / k)
        np.fill_diagonal(diags[k - 1], w)
    diags = diags.astype(ml_dtypes.bfloat16)
    return {"lt": lt, "diags": diags}


def _build():
    ops = _register_ops()
    AF = mybir.ActivationFunctionType
    ALU = mybir.AluOpType

    nc = bacc.Bacc("TRN2", target_bir_lowering=False, debug=False,
                   enable_asserts=True, num_devices=NCORES)

    f0_d = nc.dram_tensor("f0", [P, FD], dt.float32, kind="ExternalInput")
    loud_d = nc.dram_tensor("loud", [P, FD], dt.float32, kind="ExternalInput")
    mix_d = nc.dram_tensor("mix", [P, FD], dt.float32, kind="ExternalInput")
    noise_d = nc.dram_tensor("noise", [P, FD], dt.float32, kind="ExternalInput")
    lt_d = nc.dram_tensor("lt", [P, P], dt.float32, kind="ExternalInput")
    diags_d = nc.dram_tensor("diags", [H, P, P], dt.bfloat16, kind="ExternalInput")
    out_d = nc.dram_tensor("audio", [P, FD], dt.float32, kind="ExternalOutput")

    with tile.TileContext(nc) as tc, ExitStack() as ctx:
        pool = ctx.enter_context(tc.tile_pool(name="sb", bufs=1))
        vqpool = ctx.enter_context(tc.tile_pool(name="vq", bufs=2))
        sqpool = ctx.enter_context(tc.tile_pool(name="sq", bufs=2))
        hpool = ctx.enter_context(tc.tile_pool(name="hps", bufs=1, space="PSUM"))
        opool = ctx.enter_context(tc.tile_pool(name="ops", bufs=1, space="PSUM"))

        exp_bias = pool.tile([P, 1], dt.float32, tag="cbias_exp")
        nc.vector.memset(exp_bias[:], EXP_BIAS)
        zero_bias = pool.tile([P, 1], dt.float32, tag="cbias_zero")
        nc.vector.memset(zero_bias[:], 0.0)

        # ---- input DMA ----
        f0 = pool.tile([P, FD], dt.float32, tag="scr", bufs=4, name="f0")
        nc.sync.dma_start(f0[:], f0_d.ap())
        lt = pool.tile([P, P], dt.float32)
        nc.gpsimd.dma_start(lt[:], lt_d.ap())
        diags = pool.tile([P, H, P], dt.bfloat16)
        nc.gpsimd.dma_start(diags[:], diags_d.ap().rearrange("k p m -> p k m"))
        loud = pool.tile([P, FD], dt.float32, tag="loud")
        nc.scalar.dma_start(loud[:], loud_d.ap())
        mix = pool.tile([P, FD], dt.float32, tag="mix")
        nc.scalar.dma_start(mix[:], mix_d.ap())
        noise = pool.tile([P, FD], dt.float32, tag="noise")
        nc.scalar.dma_start(noise[:], noise_d.ap())

        # ---- stage 1: phase accumulation (turns) ----
        inc = pool.tile([P, FD], dt.float32, tag="scr", bufs=4, name="inc")
        nc.scalar.activation(inc[:], f0[:], AF.Exp, bias=exp_bias[:, 0:1], scale=1.0)

        local = pool.tile([P, FD], dt.float32, tag="scr", bufs=4, name="local")
        nc.vector._custom_dve(ops["CUMSUM1_ANT"], out=local[:], in0=inc[:],
                              s0=0.0, s1=0.0, imm2=0.0)

        offs_ps = opool.tile([P, 1], dt.float32, tag="offs")
        nc.tensor.matmul(offs_ps[:], lt[:], local[:, FD - 1:FD],
                         start=True, stop=True)
        offs = pool.tile([P, 1], dt.float32)
        nc.vector.tensor_copy(offs[:], offs_ps[:])

        # u1 = frac(local + offs) in one 1x custom-DVE op
        u1 = pool.tile([P, FD], dt.float32, tag="u1")
        nc.vector._custom_dve(ops["FRAC_AFF_ANT"], out=u1[:], in0=local[:],
                              s0=1.0, s1=offs[:, 0:1], imm2=MAGIC)

        # ---- epilogue prework on GpSimd (overlaps the k-loop) ----
        # audio = A*hw + B;  A = loud*mix, B = loud*noise*(1-mix)
        A = pool.tile([P, FD], dt.float32, tag="A")
        nc.gpsimd.tensor_tensor(A[:], loud[:], mix[:], ALU.mult)
        ln_ = pool.tile([P, FD], dt.float32, tag="scr", bufs=4, name="ln")
        nc.gpsimd.tensor_tensor(ln_[:], loud[:], noise[:], ALU.mult)
        lnm = pool.tile([P, FD], dt.float32, tag="lnm")
        nc.gpsimd.tensor_tensor(lnm[:], ln_[:], mix[:], ALU.mult)
        Bt = pool.tile([P, FD], dt.float32, tag="Bt")
        nc.gpsimd.tensor_tensor(Bt[:], ln_[:], lnm[:], ALU.subtract)

        # ---- k-loop in quads: v = frac(k*u1) @2x; sin on ACT or DVE poly ----
        hw = hpool.tile([P, 4, 512], dt.float32, tag="hw")
        NG = (H + QUAD - 1) // QUAD
        for g in range(NG):
            ks = [4 * g + j + 1 for j in range(QUAD) if 4 * g + j < H]
            n_act = sum(1 for k in ks if k <= M_ACT)
            vq = vqpool.tile([P, QUAD, FD], dt.float32, tag="vq")
            sq_t = sqpool.tile([P, QUAD, FD], dt.bfloat16, tag="sq")
            for j, k in enumerate(ks):
                nc.vector._custom_dve(ops["FRAC4X_ANT"], out=vq[:, j, :],
                                      in0=u1[:], s0=float(k), s1=0.0,
                                      imm2=MAGIC)
            if n_act:
                nc.scalar.activation(
                    sq_t[:, 0:n_act, :].rearrange("p a f -> p (a f)"),
                    vq[:, 0:n_act, :].rearrange("p a f -> p (a f)"),
                    AF.Sin, bias=zero_bias[:, 0:1], scale=2.0 * PI)
            for j, k in enumerate(ks):
                if j >= n_act:
                    nc.vector._custom_dve(ops["SINP7_ANT"], out=sq_t[:, j, :],
                                          in0=vq[:, j, :], s0=B1, s1=B3,
                                          imm2=B5)
            for j, k in enumerate(ks):
                for qo in range(0, FD, 512):
                    qn = min(512, FD - qo)
                    nc.tensor.matmul(hw[:, qo // 512, 0:qn],
                                     diags[:, k - 1, :], sq_t[:, j, qo:qo + qn],
                                     start=(k == 1), stop=(k == H))

        # ---- epilogue: audio = A*hw + B, then per-row peak normalize ----
        hw_flat = hw[:].rearrange("p q f -> p (q f)")[:, 0:FD]
        e1 = pool.tile([P, FD], dt.float32, tag="scr", bufs=4, name="e1")
        nc.vector.tensor_tensor(e1[:], A[:], hw_flat, ALU.mult)
        audio = pool.tile([P, FD], dt.float32, tag="scr", bufs=4, name="audio")
        nc.vector.tensor_tensor(audio[:], e1[:], Bt[:], ALU.add)

        # per-row peak: free-dim abs-max then 32x32 block transpose trick
        pk = pool.tile([P, 1], dt.float32, tag="pk")
        nc.vector.tensor_reduce(pk[:], audio[:], axis=mybir.AxisListType.X,
                                op=ALU.max, apply_absolute_value=True)
        pkr = pool.tile([P, 32], dt.float32, tag="pkr")
        nc.vector.tensor_copy(pkr[:], pk[:, 0:1].to_broadcast((P, 32)))
        pkt = pool.tile([P, 32], dt.float32, tag="pkt")
        nc.vector.transpose(pkt[:], pkr[:])
        rowmax = pool.tile([P, 1], dt.float32, tag="rowmax")
        nc.vector.tensor_reduce(rowmax[:], pkt[:],
                                axis=mybir.AxisListType.X, op=ALU.max)
        pke = pool.tile([P, 1], dt.float32, tag="pke")
        nc.vector.tensor_scalar(pke[:], rowmax[:], 1e-6, None, ALU.add)
        rcp = pool.tile([P, 1], dt.float32, tag="rcp")
        nc.vector.reciprocal(rcp[:], pke[:])
        outt = pool.tile([P, FD], dt.float32, tag="scr", bufs=4, name="outt")
        nc.vector.tensor_scalar(outt[:], audio[:], rcp[:, 0:1], None, ALU.mult)
        nc.sync.dma_start(out_d.ap(), outt[:])

    nc.compile()
    return nc


def kernel(f0, loudness, harmonic_mix, noise):
    if "nc" not in _cache:
        _cache["nc"] = _build()
        _cache["consts"] = _consts()
    nc = _cache["nc"]
    consts = _cache["consts"]

    def shard(a, c):
        return np.ascontiguousarray(
            a[c * RPC:(c + 1) * RPC].astype(f32, copy=False).reshape(P, FD))

    in_maps = []
    for c in range(NCORES):
        in_maps.append({
            "f0": shard(f0, c),
            "loud": shard(loudness, c),
            "mix": shard(harmonic_mix, c),
            "noise": shard(noise, c),
            **consts,
        })

    res = bass_utils.run_bass_kernel_spmd(nc, in_maps, core_ids=list(range(NCORES)))
    outs = [res.results[c]["audio"].reshape(RPC, T) for c in range(NCORES)]
    return np.concatenate(outs, axis=0)


# revision 4
# speedup vs baseline: 1.1232x; 1.0624x over previous
"""DDSP core synthesizer kernel for Trainium2 (8 NeuronCores, data-parallel).

Reference computation (per row of B=32, T=64000):
    f0_hz = 20*exp(f0); phase = cumsum(2*pi*f0_hz/SR)
    hw    = sum_k sin(phase*k)/k   (k = 1..60)
    audio = mix*hw*loud + (1-mix)*noise*loud;  out = audio / (max|audio| + 1e-6)

Device algorithm (phase kept in "turns"), layout [128 partitions = 4 rows x
32 blocks, 2000 free = time-in-block]:
    inc  = exp(f0 + ln(20/SR))                     [ACT Exp]
    u    = single-src cumsum (custom DVE scan); cross-block offsets via a
           triangular matmul [PE]; u1 = frac(u+offs) [custom DVE, 1x]
    per harmonic k (k=1 reuses u1 directly — frac(1*u1) = u1):
        v_k = frac(k*u1)     [custom DVE FRAC4X — hand-authored 2x_2p uop:
                              the 4-slice frac chain duplicated across the
                              8-slice pipe reading both SBUF ports, with
                              perf_max=2 in instruction byte 36 -> runs at
                              2 elem/cycle/lane, ~1.19us per [128,2000] op]
        k <= M_ACT: s_k = sin(2pi v_k) -> bf16     [ACT Sin, one op per
                              GROUP of up to 4 harmonics: same scale for
                              all k, so quads amortize the 352-cyc fixed
                              cost; groups sized 1,2,4.. so ACT starts
                              right after u1]
        k >  M_ACT: s_k = P7(v_k)/|a7| -> bf16     [custom DVE SINP7, deg-7
                              odd minimax poly (max err 2.5e-4), leading
                              coeff normalized so constants are
                              k-independent; |a7|/k goes into the PE diag.
                              Interleaved into the DVE slack while ACT is
                              the pacer.]
        hw += diag(w_k) @ s_k in PSUM              [PE; w_k = 1/k or |a7|/k]
    noise is folded in via PSUM too: hw -= I @ bf16(noise)  [PE]
    epilogue: audio = G*hw + ln with G = loud*mix, ln = loud*noise
              (both computed on the DVE inside its slack windows; GpSimd is
              kept DMA-only — its tensor ops starve the 2x frac of SBUF
              ports); per-row peak normalize (free-reduce + 32x32 transpose).

Sharding: pure data parallel, 4 rows per core, SPMD on cores 0-7.
"""

import sys

sys.path.insert(0, "/opt/trn_rl_repo")

import numpy as np
import ml_dtypes
from contextlib import ExitStack

import concourse.bass as bass
import concourse.tile as tile
from concourse import bacc, mybir
from concourse import bass_utils

f32 = np.float32
dt = mybir.dt

SR = 44100.0
H = 60                      # harmonics
B, T = 32, 64000
NCORES = 8
RPC = B // NCORES           # rows per core = 4
P = 128                     # SBUF partitions
FD = T * RPC // P           # free dim of master tiles = 2000
BPR = P // RPC              # blocks per row = 32
PI = float(np.pi)
MAGIC = float(1.5 * 2.0 ** 23)
EXP_BIAS = float(np.log(20.0 / SR))

# deg-7 odd minimax fit of sin(2*pi*f) on f in [-0.5, 0.5]; max err 2.5e-4
A1, A3, A5, A7 = 6.27863591, -41.09374848, 77.93051701, 56.08683302
B1, B3, B5 = A1 / A7, A3 / A7, A5 / A7     # P = f(B1+y(B3+y(B5-y))) = sin/|a7|

M_ACT = 52                  # harmonics 1..M_ACT on ACT Sin; rest on DVE poly

_cache = {}


def _register_ops():
    """Custom DVE ops. FRAC4X additionally carries a hand-written 2x_2p
    (two-read-port) uop program + perf_max=2 so the engine runs it at
    2 elem/cycle/lane for fp32 SBUF single-source calls."""
    if "ops" in _cache:
        return _cache["ops"]
    from concourse import dve_ops
    from concourse.dve_spec import (Spec, Src0, C0, C1, C2, lower, scan, sq,
                                    AluOp)
    from concourse.dve_spec import _has_src1 as has_src1
    from concourse.dve_uop import (DveOpSpec, UopConfig, InpSel, AluInp,
                                   OutSel, OutPath, Trigger, ENABLE, DelayInp)
    from concourse.dve_uop import AluOp as UAluOp
    from concourse.dve_table_gen import dve_ver_for

    ver = dve_ver_for("TRN2")
    ops = {}

    def base_reg(name, spec, myspec=None):
        if name not in dve_ops._SUB_OPCODE_FOR_NAME:
            row = max(dve_ops._SUB_OPCODE_FOR_NAME.values()) + 1
            assert row < 0x20
            dve_ops._SUB_OPCODE_FOR_NAME[name] = row
        if myspec is None:
            myspec_ = DveOpSpec(
                name=name, opcode=dve_ops.get_dve_sub_opcode(name),
                uops=lower(spec, ver=ver), rd1_en=has_src1(spec))
        else:
            myspec_ = myspec
            myspec_.opcode = dve_ops.get_dve_sub_opcode(name)
            myspec_.validate(ver)

        class _Op(dve_ops.DveOp):
            def compile(self, ver_):
                return myspec_

        op = _Op(name, spec, subdim=False, uops_sha={ver: myspec_.sha(ver)})
        if not any(o.name == name for o in dve_ops.OPS):
            dve_ops.OPS.append(op)
        dve_ops.CUSTOM_DVE_SPECS[name] = spec
        ops[name] = op
        return op

    # ---- FRAC_AFFINE (1x, 5 slices): f = x - rint(x), x = Src0*C0 + C1 ----
    def fa_ref(in0, in1, s0, s1, imm2):
        x = (in0.astype(f32) * f32(s0) + f32(s1)).astype(f32)
        w = ((x + f32(imm2)).astype(f32) - f32(imm2)).astype(f32)
        return (x - w).astype(f32)
    xa = Src0 * C0 + C1
    base_reg("FRAC_AFF_ANT", Spec(body=xa - ((xa + C2) - C2), reference=fa_ref))

    # ---- CUMSUM1 (1x, single-source scan) ----
    def cs_ref(in0, in1, s0, s1, imm2):
        return np.cumsum(in0.astype(f32), axis=-1, dtype=f32)
    base_reg("CUMSUM1_ANT", Spec(body=scan(AluOp.ADD, Src0), reference=cs_ref))

    # ---- SINP7 (1x, 7 slices): out = Src0*(C0 + y*(C1 + y*(C2 - y))) ----
    def sp_ref(in0, in1, s0, s1, imm2):
        f = in0.astype(f32)
        y = (f * f).astype(f32)
        t = (f32(imm2) - y).astype(f32)
        t = (y * t).astype(f32)
        t = (t + f32(s1)).astype(f32)
        t = (y * t).astype(f32)
        t = (t + f32(s0)).astype(f32)
        return (f * t).astype(f32)
    yq = sq(Src0)
    base_reg("SINP7_ANT", Spec(body=Src0 * (C0 + yq * (C1 + yq * (C2 - yq))),
                               reference=sp_ref))

    # ---- FRAC4X (2x_2p): f = x - rint(x), x = Src0*C0 ----
    def f4_ref(in0, in1, s0, s1, imm2):
        x = (in0.astype(f32) * f32(s0)).astype(f32)
        w = ((x + f32(imm2)).astype(f32) - f32(imm2)).astype(f32)
        return (x - w).astype(f32)
    xb = Src0 * C0
    spec4 = Spec(body=xb - ((xb + C2) - C2), reference=f4_ref)
    uops_1x = lower(spec4, ver=ver)

    u = UopConfig()
    u.enable_input(InpSel.SRC_0, 1)    # chain 0: element i  (read port 0)
    u.enable_input(InpSel.CONST_0, 2)  # chain 1: k
    u.enable_input(InpSel.CONST_2, 3)  # chain 2: magic rint constant
    u.enable_input(InpSel.SRC_1, 4)    # chain 3: element i+1 (read port 1)
    u.require_inp0 = ENABLE
    u.require_inp1 = ENABLE
    u.trigger = (Trigger.SRC_TENSOR_DONE, Trigger.NONE, Trigger.NONE)
    b = u.datapath_config
    PD, PA = DelayInp.PREV_DELAY, DelayInp.PREV_ALU_OUT
    # chain A (elem i): blocks 0-3; chain B (elem i+1): blocks 4-7
    b[0].enable_alu(UAluOp.MULTIPLY, AluInp.PREV_DELAY_0, AluInp.PREV_DELAY_1)
    for c in (0, 1, 2, 3):
        b[0].enable_delay_from_src(PD, c)
    b[1].enable_alu(UAluOp.ADD, AluInp.PREV_ALU_OUT, AluInp.PREV_DELAY_2)
    b[1].enable_delay_from_src(PA, 0)
    for c in (1, 2, 3):
        b[1].enable_delay_from_src(PD, c)
    b[2].enable_alu(UAluOp.SUBTRACT, AluInp.PREV_ALU_OUT, AluInp.PREV_DELAY_2)
    for c in (0, 1, 2, 3):
        b[2].enable_delay_from_src(PD, c)
    b[3].enable_alu(UAluOp.SUBTRACT, AluInp.PREV_DELAY_0, AluInp.PREV_ALU_OUT)
    for c in (1, 2, 3):
        b[3].enable_delay_from_src(PD, c)
    b[4].enable_alu(UAluOp.MULTIPLY, AluInp.PREV_DELAY_3, AluInp.PREV_DELAY_1)
    b[4].enable_delay_from_src(PA, 0)
    b[4].enable_delay_from_src(PD, 2)
    b[5].enable_alu(UAluOp.ADD, AluInp.PREV_ALU_OUT, AluInp.PREV_DELAY_2)
    b[5].enable_delay_from_src(PA, 3)
    for c in (0, 2):
        b[5].enable_delay_from_src(PD, c)
    b[6].enable_alu(UAluOp.SUBTRACT, AluInp.PREV_ALU_OUT, AluInp.PREV_DELAY_2)
    for c in (0, 3):
        b[6].enable_delay_from_src(PD, c)
    b[7].enable_alu(UAluOp.SUBTRACT, AluInp.PREV_DELAY_3, AluInp.PREV_ALU_OUT)
    b[7].enable_delay_from_src(PD, 0)
    u.enable_output(OutSel.DELAY_0, OutPath.WR0_LO)   # f(elem i)
    u.enable_output(OutSel.ALU_OUT, OutPath.WR1_LO)   # f(elem i+1)

    spec4x = DveOpSpec(
        name="FRAC4X_ANT", uops=uops_1x, uops_2x=[uops_1x[0]],
        uops_2x_2p=[u], uops_4x=None, perf_max=2, rd1_en=has_src1(spec4))
    base_reg("FRAC4X_ANT", spec4, myspec=spec4x)

    _patch_perf_max({"FRAC4X_ANT": 2})
    _cache["ops"] = ops
    return ops


def _patch_perf_max(pm_ops):
    """_custom_dve builds InstCustomDveAnt without perf_max (byte-36[7:6]) and
    add_instruction stores a copy, so swap the constructor symbol for a
    factory that injects it for our 2x-capable ops."""
    from concourse import bass_isa
    real = mybir.InstCustomDveAnt
    if getattr(bass_isa.InstCustomDveAnt, "_pm_patched", False):
        return

    def make(*args, **kw):
        pm = pm_ops.get(kw.get("op_name"))
        if pm is not None:
            kw.setdefault("perf_max", pm)
        return real(*args, **kw)

    make._pm_patched = True
    bass_isa.InstCustomDveAnt = make


def _consts():
    # lt: exclusive-prefix matmul weights. offs[m] = sum_k lt[k, m] * totals[k]
    kk, mm_ = np.meshgrid(np.arange(P), np.arange(P), indexing="ij")
    lt = ((kk // BPR == mm_ // BPR) & (kk % BPR < mm_ % BPR)).astype(f32)

    # diags[k-1] = diag(w_k): w_k = 1/k (ACT sin) or |a7|/k (normalized DVE
    # poly).  diags[H] = -I for the noise fold-in.
    diags = np.zeros((H + 1, P, P), dtype=np.float64)
    for k in range(1, H + 1):
        w = (1.0 / k) if k <= M_ACT else (A7 / k)
        np.fill_diagonal(diags[k - 1], w)
    np.fill_diagonal(diags[H], -1.0)
    diags = diags.astype(ml_dtypes.bfloat16)
    return {"lt": lt, "diags": diags}


def _build():
    ops = _register_ops()
    AF = mybir.ActivationFunctionType
    ALU = mybir.AluOpType

    nc = bacc.Bacc("TRN2", target_bir_lowering=False, debug=False,
                   enable_asserts=True, num_devices=NCORES)

    f0_d = nc.dram_tensor("f0", [P, FD], dt.float32, kind="ExternalInput")
    loud_d = nc.dram_tensor("loud", [P, FD], dt.float32, kind="ExternalInput")
    mix_d = nc.dram_tensor("mix", [P, FD], dt.float32, kind="ExternalInput")
    noise_d = nc.dram_tensor("noise", [P, FD], dt.float32, kind="ExternalInput")
    lt_d = nc.dram_tensor("lt", [P, P], dt.float32, kind="ExternalInput")
    diags_d = nc.dram_tensor("diags", [H + 1, P, P], dt.bfloat16,
                             kind="ExternalInput")
    out_d = nc.dram_tensor("audio", [P, FD], dt.float32, kind="ExternalOutput")

    # ACT groups (k values per Sin op) then DVE-poly harmonics
    groups = [[1], [2, 3]]
    k = 4
    while k + 3 <= M_ACT:
        groups.append([k, k + 1, k + 2, k + 3])
        k += 4
    if k <= M_ACT:
        groups.append(list(range(k, M_ACT + 1)))
    dve_ks = list(range(M_ACT + 1, H + 1))
    # DVE-stream inserts after group index i (runs inside the ACT-paced slack)
    inserts = {3: ["nbf"], 4: ["G"], 5: ["ln"]}
    gi = 6
    for dk in dve_ks:
        inserts.setdefault(gi, []).append(dk)
        gi += 1
        if gi >= len(groups) - 1:
            gi = 6  # wrap (shouldn't happen with default sizes)

    with tile.TileContext(nc) as tc, ExitStack() as ctx:
        pool = ctx.enter_context(tc.tile_pool(name="sb", bufs=1))
        vqpool = ctx.enter_context(tc.tile_pool(name="vq", bufs=2))
        sqpool = ctx.enter_context(tc.tile_pool(name="sq", bufs=2))
        vdpool = ctx.enter_context(tc.tile_pool(name="vd", bufs=1))
        sdpool = ctx.enter_context(tc.tile_pool(name="sd", bufs=1))
        hpool = ctx.enter_context(tc.tile_pool(name="hps", bufs=1, space="PSUM"))
        opool = ctx.enter_context(tc.tile_pool(name="ops", bufs=1, space="PSUM"))

        exp_bias = pool.tile([P, 1], dt.float32, tag="cbias_exp")
        nc.vector.memset(exp_bias[:], EXP_BIAS)
        zero_bias = pool.tile([P, 1], dt.float32, tag="cbias_zero")
        nc.vector.memset(zero_bias[:], 0.0)

        # ---- input DMA (GpSimd queues are DMA-only) ----
        f0 = pool.tile([P, FD], dt.float32, tag="scr", bufs=4, name="f0")
        nc.sync.dma_start(f0[:], f0_d.ap())
        lt = pool.tile([P, P], dt.float32)
        nc.gpsimd.dma_start(lt[:], lt_d.ap())
        diags = pool.tile([P, H + 1, P], dt.bfloat16)
        nc.gpsimd.dma_start(diags[:], diags_d.ap().rearrange("k p m -> p k m"))
        loud = pool.tile([P, FD], dt.float32, tag="loud")
        nc.scalar.dma_start(loud[:], loud_d.ap())
        mix = pool.tile([P, FD], dt.float32, tag="mix")
        nc.scalar.dma_start(mix[:], mix_d.ap())
        noise = pool.tile([P, FD], dt.float32, tag="noise")
        nc.scalar.dma_start(noise[:], noise_d.ap())

        # ---- stage 1: phase accumulation (turns) ----
        inc = pool.tile([P, FD], dt.float32, tag="scr", bufs=4, name="inc")
        nc.scalar.activation(inc[:], f0[:], AF.Exp, bias=exp_bias[:, 0:1],
                             scale=1.0)

        local = pool.tile([P, FD], dt.float32, tag="scr", bufs=4, name="local")
        nc.vector._custom_dve(ops["CUMSUM1_ANT"], out=local[:], in0=inc[:],
                              s0=0.0, s1=0.0, imm2=0.0)

        offs_ps = opool.tile([P, 1], dt.float32, tag="offs")
        nc.tensor.matmul(offs_ps[:], lt[:], local[:, FD - 1:FD],
                         start=True, stop=True)
        offs = pool.tile([P, 1], dt.float32)
        nc.vector.tensor_copy(offs[:], offs_ps[:])

        u1 = pool.tile([P, FD], dt.float32, tag="u1")
        nc.vector._custom_dve(ops["FRAC_AFF_ANT"], out=u1[:], in0=local[:],
                              s0=1.0, s1=offs[:, 0:1], imm2=MAGIC)

        # epilogue operand tiles (filled inside the k-loop slack)
        G = pool.tile([P, FD], dt.float32, tag="G")
        ln_ = pool.tile([P, FD], dt.float32, tag="ln")
        nbf = pool.tile([P, FD], dt.bfloat16, tag="nbf")

        # ---- k-loop ----
        hw = hpool.tile([P, 4, 512], dt.float32, tag="hw")

        def emit_mms(k_, s_ap, start, stop):
            for qo in range(0, FD, 512):
                qn = min(512, FD - qo)
                nc.tensor.matmul(hw[:, qo // 512, 0:qn],
                                 diags[:, k_, :], s_ap[:, qo:qo + qn],
                                 start=start, stop=stop)

        last_group = len(groups) - 1
        for gidx, ks in enumerate(groups):
            n = len(ks)
            if ks == [1]:
                s_t = sqpool.tile([P, 4, FD], dt.bfloat16, tag="sq")
                nc.scalar.activation(s_t[:, 0, :], u1[:], AF.Sin,
                                     bias=zero_bias[:, 0:1], scale=2.0 * PI)
                emit_mms(0, s_t[:, 0, :], True, False)
            else:
                vq = vqpool.tile([P, 4, FD], dt.float32, tag="vq")
                s_t = sqpool.tile([P, 4, FD], dt.bfloat16, tag="sq")
                for j, kk_ in enumerate(ks):
                    nc.vector._custom_dve(ops["FRAC4X_ANT"], out=vq[:, j, :],
                                          in0=u1[:], s0=float(kk_), s1=0.0,
                                          imm2=MAGIC)
                nc.scalar.activation(
                    s_t[:, 0:n, :].rearrange("p a f -> p (a f)"),
                    vq[:, 0:n, :].rearrange("p a f -> p (a f)"),
                    AF.Sin, bias=zero_bias[:, 0:1], scale=2.0 * PI)
                for j, kk_ in enumerate(ks):
                    emit_mms(kk_ - 1, s_t[:, j, :], False,
                             gidx == last_group and j == n - 1)

            for ins in inserts.get(gidx, []):
                if ins == "G":
                    nc.vector.tensor_tensor(G[:], loud[:], mix[:], ALU.mult)
                elif ins == "ln":
                    nc.vector.tensor_tensor(ln_[:], loud[:], noise[:], ALU.mult)
                elif ins == "nbf":
                    nc.vector.tensor_copy(nbf[:], noise[:])
                else:  # a DVE-poly harmonic
                    dk = ins
                    vd = vdpool.tile([P, FD], dt.float32, tag="vd")
                    nc.vector._custom_dve(ops["FRAC4X_ANT"], out=vd[:],
                                          in0=u1[:], s0=float(dk), s1=0.0,
                                          imm2=MAGIC)
                    sd = sdpool.tile([P, FD], dt.bfloat16, tag="sd")
                    nc.vector._custom_dve(ops["SINP7_ANT"], out=sd[:],
                                          in0=vd[:], s0=B1, s1=B3, imm2=B5)
                    emit_mms(dk - 1, sd[:], False, False)
                    if dk == dve_ks[-1]:
                        # noise fold-in: hw -= I @ bf16(noise)
                        emit_mms(H, nbf[:], False, False)

        # ---- epilogue: audio = G*hw + ln, then per-row peak normalize ----
        hw_flat = hw[:].rearrange("p q f -> p (q f)")[:, 0:FD]
        e1 = pool.tile([P, FD], dt.float32, tag="scr", bufs=4, name="e1")
        nc.vector.tensor_tensor(e1[:], G[:], hw_flat, ALU.mult)
        audio = pool.tile([P, FD], dt.float32, tag="scr", bufs=4, name="audio")
        nc.vector.tensor_tensor(audio[:], e1[:], ln_[:], ALU.add)

        pk = pool.tile([P, 1], dt.float32, tag="pk")
        nc.vector.tensor_reduce(pk[:], audio[:], axis=mybir.AxisListType.X,
                                op=ALU.max, apply_absolute_value=True)
        pkr = pool.tile([P, 32], dt.float32, tag="pkr")
        nc.vector.tensor_copy(pkr[:], pk[:, 0:1].to_broadcast((P, 32)))
        pkt = pool.tile([P, 32], dt.float32, tag="pkt")
        nc.vector.transpose(pkt[:], pkr[:])
        rowmax = pool.tile([P, 1], dt.float32, tag="rowmax")
        nc.vector.tensor_reduce(rowmax[:], pkt[:],
                                axis=mybir.AxisListType.X, op=ALU.max)
        pke = pool.tile([P, 1], dt.float32, tag="pke")
        nc.vector.tensor_scalar(pke[:], rowmax[:], 1e-6, None, ALU.add)
        rcp = pool.tile([P, 1], dt.float32, tag="rcp")
        nc.vector.reciprocal(rcp[:], pke[:])
        outt = pool.tile([P, FD], dt.float32, tag="scr", bufs=4, name="outt")
        nc.vector.tensor_scalar(outt[:], audio[:], rcp[:, 0:1], None, ALU.mult)
        nc.sync.dma_start(out_d.ap(), outt[:])

    nc.compile()
    return nc


def kernel(f0, loudness, harmonic_mix, noise):
    if "nc" not in _cache:
        _cache["nc"] = _build()
        _cache["consts"] = _consts()
    nc = _cache["nc"]
    consts = _cache["consts"]

    def shard(a, c):
        return np.ascontiguousarray(
            a[c * RPC:(c + 1) * RPC].astype(f32, copy=False).reshape(P, FD))

    in_maps = []
    for c in range(NCORES):
        in_maps.append({
            "f0": shard(f0, c),
            "loud": shard(loudness, c),
            "mix": shard(harmonic_mix, c),
            "noise": shard(noise, c),
            **consts,
        })

    res = bass_utils.run_bass_kernel_spmd(nc, in_maps, core_ids=list(range(NCORES)))
    outs = [res.results[c]["audio"].reshape(RPC, T) for c in range(NCORES)]
    return np.concatenate(outs, axis=0)


# revision 8
# speedup vs baseline: 1.1258x; 1.0023x over previous
"""DDSP core synthesizer kernel for Trainium2 (8 NeuronCores, data-parallel).

Reference computation (per row of B=32, T=64000):
    f0_hz = 20*exp(f0); phase = cumsum(2*pi*f0_hz/SR)
    hw    = sum_k sin(phase*k)/k   (k = 1..60)
    audio = mix*hw*loud + (1-mix)*noise*loud;  out = audio / (max|audio| + 1e-6)

Device algorithm (phase kept in "turns"), layout [128 partitions = 4 rows x
32 blocks, 2000 free = time-in-block]:
    inc  = exp(f0 + ln(20/SR))                     [ACT Exp]
    u    = single-src cumsum (custom DVE scan); cross-block offsets via a
           triangular matmul [PE]; u1 = frac(u+offs) [custom DVE, 1x]
    per harmonic k (k=1 reuses u1 directly — frac(1*u1) = u1):
        v_k = frac(k*u1)     [custom DVE FRAC4X — hand-authored 2x_2p uop:
                              the 4-slice frac chain duplicated across the
                              8-slice pipe reading both SBUF ports, with
                              perf_max=2 in instruction byte 36 -> runs at
                              2 elem/cycle/lane, ~1.19us per [128,2000] op]
        k <= M_ACT: s_k = sin(2pi v_k) -> bf16     [ACT Sin, one op per
                              GROUP of up to 4 harmonics: same scale for
                              all k, so quads amortize the 352-cyc fixed
                              cost; groups sized 1,2,4.. so ACT starts
                              right after u1]
        k >  M_ACT: s_k = P7(v_k)/|a7| -> bf16     [custom DVE SINP7, deg-7
                              odd minimax poly (max err 2.5e-4), leading
                              coeff normalized so constants are
                              k-independent; |a7|/k goes into the PE diag.
                              Interleaved into the DVE slack while ACT is
                              the pacer.]
        hw += diag(w_k) @ s_k in PSUM              [PE; w_k = 1/k or |a7|/k]
    noise is folded in via PSUM too: hw -= I @ bf16(noise)  [PE]
    epilogue: audio = G*hw + ln with G = loud*mix, ln = loud*noise
              (both computed on the DVE inside its slack windows; GpSimd is
              kept DMA-only — its tensor ops starve the 2x frac of SBUF
              ports); per-row peak normalize (free-reduce + 32x32 transpose).

Sharding: pure data parallel, 4 rows per core, SPMD on cores 0-7.
"""

import sys

sys.path.insert(0, "/opt/trn_rl_repo")

import numpy as np
import ml_dtypes
from contextlib import ExitStack

import concourse.bass as bass
import concourse.tile as tile
from concourse import bacc, mybir
from concourse import bass_utils

f32 = np.float32
dt = mybir.dt

SR = 44100.0
H = 60                      # harmonics
B, T = 32, 64000
NCORES = 8
RPC = B // NCORES           # rows per core = 4
P = 128                     # SBUF partitions
FD = T * RPC // P           # free dim of master tiles = 2000
BPR = P // RPC              # blocks per row = 32
PI = float(np.pi)
MAGIC = float(1.5 * 2.0 ** 23)
EXP_BIAS = float(np.log(20.0 / SR))

# deg-7 odd minimax fit of sin(2*pi*f) on f in [-0.5, 0.5]; max err 2.5e-4
A1, A3, A5, A7 = 6.27863591, -41.09374848, 77.93051701, 56.08683302
B1, B3, B5 = A1 / A7, A3 / A7, A5 / A7     # P = f(B1+y(B3+y(B5-y))) = sin/|a7|

M_ACT = 54                  # harmonics 1..M_ACT on ACT Sin; rest on DVE poly

_cache = {}


def _register_ops():
    """Custom DVE ops. FRAC4X additionally carries a hand-written 2x_2p
    (two-read-port) uop program + perf_max=2 so the engine runs it at
    2 elem/cycle/lane for fp32 SBUF single-source calls."""
    if "ops" in _cache:
        return _cache["ops"]
    from concourse import dve_ops
    from concourse.dve_spec import (Spec, Src0, C0, C1, C2, lower, scan, sq,
                                    AluOp)
    from concourse.dve_spec import _has_src1 as has_src1
    from concourse.dve_uop import (DveOpSpec, UopConfig, InpSel, AluInp,
                                   OutSel, OutPath, Trigger, ENABLE, DelayInp)
    from concourse.dve_uop import AluOp as UAluOp
    from concourse.dve_table_gen import dve_ver_for

    ver = dve_ver_for("TRN2")
    ops = {}

    def base_reg(name, spec, myspec=None):
        if name not in dve_ops._SUB_OPCODE_FOR_NAME:
            row = max(dve_ops._SUB_OPCODE_FOR_NAME.values()) + 1
            assert row < 0x20
            dve_ops._SUB_OPCODE_FOR_NAME[name] = row
        if myspec is None:
            myspec_ = DveOpSpec(
                name=name, opcode=dve_ops.get_dve_sub_opcode(name),
                uops=lower(spec, ver=ver), rd1_en=has_src1(spec))
        else:
            myspec_ = myspec
            myspec_.opcode = dve_ops.get_dve_sub_opcode(name)
            myspec_.validate(ver)

        class _Op(dve_ops.DveOp):
            def compile(self, ver_):
                return myspec_

        op = _Op(name, spec, subdim=False, uops_sha={ver: myspec_.sha(ver)})
        if not any(o.name == name for o in dve_ops.OPS):
            dve_ops.OPS.append(op)
        dve_ops.CUSTOM_DVE_SPECS[name] = spec
        ops[name] = op
        return op

    # ---- FRAC_AFFINE (1x, 5 slices): f = x - rint(x), x = Src0*C0 + C1 ----
    def fa_ref(in0, in1, s0, s1, imm2):
        x = (in0.astype(f32) * f32(s0) + f32(s1)).astype(f32)
        w = ((x + f32(imm2)).astype(f32) - f32(imm2)).astype(f32)
        return (x - w).astype(f32)
    xa = Src0 * C0 + C1
    base_reg("FRAC_AFF_ANT", Spec(body=xa - ((xa + C2) - C2), reference=fa_ref))

    # ---- CUMSUM1 (1x, single-source scan) ----
    def cs_ref(in0, in1, s0, s1, imm2):
        return np.cumsum(in0.astype(f32), axis=-1, dtype=f32)
    base_reg("CUMSUM1_ANT", Spec(body=scan(AluOp.ADD, Src0), reference=cs_ref))

    # ---- SINP7 (1x, 7 slices): out = Src0*(C0 + y*(C1 + y*(C2 - y))) ----
    def sp_ref(in0, in1, s0, s1, imm2):
        f = in0.astype(f32)
        y = (f * f).astype(f32)
        t = (f32(imm2) - y).astype(f32)
        t = (y * t).astype(f32)
        t = (t + f32(s1)).astype(f32)
        t = (y * t).astype(f32)
        t = (t + f32(s0)).astype(f32)
        return (f * t).astype(f32)
    yq = sq(Src0)
    base_reg("SINP7_ANT", Spec(body=Src0 * (C0 + yq * (C1 + yq * (C2 - yq))),
                               reference=sp_ref))

    # ---- FRAC4X (2x_2p): f = x - rint(x), x = Src0*C0 ----
    def f4_ref(in0, in1, s0, s1, imm2):
        x = (in0.astype(f32) * f32(s0)).astype(f32)
        w = ((x + f32(imm2)).astype(f32) - f32(imm2)).astype(f32)
        return (x - w).astype(f32)
    xb = Src0 * C0
    spec4 = Spec(body=xb - ((xb + C2) - C2), reference=f4_ref)
    uops_1x = lower(spec4, ver=ver)

    u = UopConfig()
    u.enable_input(InpSel.SRC_0, 1)    # chain 0: element i  (read port 0)
    u.enable_input(InpSel.CONST_0, 2)  # chain 1: k
    u.enable_input(InpSel.CONST_2, 3)  # chain 2: magic rint constant
    u.enable_input(InpSel.SRC_1, 4)    # chain 3: element i+1 (read port 1)
    u.require_inp0 = ENABLE
    u.require_inp1 = ENABLE
    u.trigger = (Trigger.SRC_TENSOR_DONE, Trigger.NONE, Trigger.NONE)
    b = u.datapath_config
    PD, PA = DelayInp.PREV_DELAY, DelayInp.PREV_ALU_OUT
    # chain A (elem i): blocks 0-3; chain B (elem i+1): blocks 4-7
    b[0].enable_alu(UAluOp.MULTIPLY, AluInp.PREV_DELAY_0, AluInp.PREV_DELAY_1)
    for c in (0, 1, 2, 3):
        b[0].enable_delay_from_src(PD, c)
    b[1].enable_alu(UAluOp.ADD, AluInp.PREV_ALU_OUT, AluInp.PREV_DELAY_2)
    b[1].enable_delay_from_src(PA, 0)
    for c in (1, 2, 3):
        b[1].enable_delay_from_src(PD, c)
    b[2].enable_alu(UAluOp.SUBTRACT, AluInp.PREV_ALU_OUT, AluInp.PREV_DELAY_2)
    for c in (0, 1, 2, 3):
        b[2].enable_delay_from_src(PD, c)
    b[3].enable_alu(UAluOp.SUBTRACT, AluInp.PREV_DELAY_0, AluInp.PREV_ALU_OUT)
    for c in (1, 2, 3):
        b[3].enable_delay_from_src(PD, c)
    b[4].enable_alu(UAluOp.MULTIPLY, AluInp.PREV_DELAY_3, AluInp.PREV_DELAY_1)
    b[4].enable_delay_from_src(PA, 0)
    b[4].enable_delay_from_src(PD, 2)
    b[5].enable_alu(UAluOp.ADD, AluInp.PREV_ALU_OUT, AluInp.PREV_DELAY_2)
    b[5].enable_delay_from_src(PA, 3)
    for c in (0, 2):
        b[5].enable_delay_from_src(PD, c)
    b[6].enable_alu(UAluOp.SUBTRACT, AluInp.PREV_ALU_OUT, AluInp.PREV_DELAY_2)
    for c in (0, 3):
        b[6].enable_delay_from_src(PD, c)
    b[7].enable_alu(UAluOp.SUBTRACT, AluInp.PREV_DELAY_3, AluInp.PREV_ALU_OUT)
    b[7].enable_delay_from_src(PD, 0)
    u.enable_output(OutSel.DELAY_0, OutPath.WR0_LO)   # f(elem i)
    u.enable_output(OutSel.ALU_OUT, OutPath.WR1_LO)   # f(elem i+1)

    spec4x = DveOpSpec(
        name="FRAC4X_ANT", uops=uops_1x, uops_2x=[uops_1x[0]],
        uops_2x_2p=[u], uops_4x=None, perf_max=2, rd1_en=has_src1(spec4))
    base_reg("FRAC4X_ANT", spec4, myspec=spec4x)

    _patch_perf_max({"FRAC4X_ANT": 2})
    _cache["ops"] = ops
    return ops


def _patch_perf_max(pm_ops):
    """_custom_dve builds InstCustomDveAnt without perf_max (byte-36[7:6]) and
    add_instruction stores a copy, so swap the constructor symbol for a
    factory that injects it for our 2x-capable ops."""
    from concourse import bass_isa
    real = mybir.InstCustomDveAnt
    if getattr(bass_isa.InstCustomDveAnt, "_pm_patched", False):
        return

    def make(*args, **kw):
        pm = pm_ops.get(kw.get("op_name"))
        if pm is not None:
            kw.setdefault("perf_max", pm)
        return real(*args, **kw)

    make._pm_patched = True
    bass_isa.InstCustomDveAnt = make


def _consts():
    # lt: exclusive-prefix matmul weights. offs[m] = sum_k lt[k, m] * totals[k]
    kk, mm_ = np.meshgrid(np.arange(P), np.arange(P), indexing="ij")
    lt = ((kk // BPR == mm_ // BPR) & (kk % BPR < mm_ % BPR)).astype(f32)

    # diags[k-1] = diag(w_k): w_k = 1/k (ACT sin) or |a7|/k (normalized DVE
    # poly).  diags[H] = -I for the noise fold-in.  Stored host-side already
    # transposed to [P, H+1, P] so the device DMA is fully contiguous (the
    # strided rearrange DMA clogged the DMA engines for ~23us).
    diags = np.zeros((H + 1, P, P), dtype=np.float64)
    for k in range(1, H + 1):
        w = (1.0 / k) if k <= M_ACT else (A7 / k)
        np.fill_diagonal(diags[k - 1], w)
    np.fill_diagonal(diags[H], -1.0)
    diags = np.ascontiguousarray(
        diags.transpose(1, 0, 2)).astype(ml_dtypes.bfloat16)
    return {"lt": lt, "diags": diags}


def _build():
    ops = _register_ops()
    AF = mybir.ActivationFunctionType
    ALU = mybir.AluOpType

    nc = bacc.Bacc("TRN2", target_bir_lowering=False, debug=False,
                   enable_asserts=True, num_devices=NCORES)

    from bass_rust import add_dep_helper

    f0_d = nc.dram_tensor("f0", [P, FD], dt.float32, kind="ExternalInput")
    loud_d = nc.dram_tensor("loud", [P, FD], dt.float32, kind="ExternalInput")
    mix_d = nc.dram_tensor("mix", [P, FD], dt.float32, kind="ExternalInput")
    noise_d = nc.dram_tensor("noise", [P, FD], dt.float32, kind="ExternalInput")
    lt_d = nc.dram_tensor("lt", [P, P], dt.float32, kind="ExternalInput")
    diags_d = nc.dram_tensor("diags", [P, H + 1, P], dt.bfloat16,
                             kind="ExternalInput")
    out_d = nc.dram_tensor("audio", [P, FD], dt.float32, kind="ExternalOutput")

    # ACT groups (k values per Sin op) then DVE-poly harmonics
    groups = [[1], [2, 3]]
    k = 4
    while k + 3 <= M_ACT:
        groups.append([k, k + 1, k + 2, k + 3])
        k += 4
    if k <= M_ACT:
        groups.append(list(range(k, M_ACT + 1)))
    dve_ks = list(range(M_ACT + 1, H + 1))
    # DVE-stream inserts after group index i (runs inside the ACT-paced slack)
    inserts = {3: ["G"], 4: ["ln"]}
    gi = 5
    for dk in dve_ks:
        inserts.setdefault(gi, []).append(dk)
        gi += 1

    # per-engine in-order chaining: pin the static schedule to emission order
    # (the Tile list-scheduler otherwise hoists ready-early ops, e.g. the
    # epilogue prework, ahead of the phase pipeline)
    prev_i = {}

    def chain(eng, bi):
        p = prev_i.get(eng)
        if p is not None:
            add_dep_helper(bi.ins, p.ins, sync=False, reason="stream-order")
        prev_i[eng] = bi
        return bi

    with tile.TileContext(nc) as tc, ExitStack() as ctx:
        pool = ctx.enter_context(tc.tile_pool(name="sb", bufs=1))
        vqpool = ctx.enter_context(tc.tile_pool(name="vq", bufs=2))
        sqpool = ctx.enter_context(tc.tile_pool(name="sq", bufs=2))
        vdpool = ctx.enter_context(tc.tile_pool(name="vd", bufs=1))
        sdpool = ctx.enter_context(tc.tile_pool(name="sd", bufs=1))
        hpool = ctx.enter_context(tc.tile_pool(name="hps", bufs=1, space="PSUM"))
        opool = ctx.enter_context(tc.tile_pool(name="ops", bufs=1, space="PSUM"))

        exp_bias = pool.tile([P, 1], dt.float32, tag="cbias_exp")
        chain("v", nc.vector.memset(exp_bias[:], EXP_BIAS))
        zero_bias = pool.tile([P, 1], dt.float32, tag="cbias_zero")
        chain("v", nc.vector.memset(zero_bias[:], 0.0))

        # ---- input DMA (GpSimd queues are DMA-only) ----
        f0 = pool.tile([P, FD], dt.float32, tag="scr", bufs=4, name="f0")
        nc.sync.dma_start(f0[:], f0_d.ap())
        lt = pool.tile([P, P], dt.float32)
        nc.gpsimd.dma_start(lt[:], lt_d.ap())
        diags = pool.tile([P, H + 1, P], dt.bfloat16)
        for ko in range(0, H + 1, 16):
            kn = min(16, H + 1 - ko)
            nc.gpsimd.dma_start(diags[:, ko:ko + kn, :],
                                diags_d.ap()[:, ko:ko + kn, :])
        loud = pool.tile([P, FD], dt.float32, tag="loud")
        nc.scalar.dma_start(loud[:], loud_d.ap())
        mix = pool.tile([P, FD], dt.float32, tag="mix")
        nc.scalar.dma_start(mix[:], mix_d.ap())
        noise = pool.tile([P, FD], dt.float32, tag="noise")
        nc.scalar.dma_start(noise[:], noise_d.ap())

        # ---- stage 1: phase accumulation (turns) ----
        inc = pool.tile([P, FD], dt.float32, tag="scr", bufs=4, name="inc")
        chain("s", nc.scalar.activation(inc[:], f0[:], AF.Exp,
                                        bias=exp_bias[:, 0:1], scale=1.0))

        local = pool.tile([P, FD], dt.float32, tag="scr", bufs=4, name="local")
        chain("v", nc.vector._custom_dve(ops["CUMSUM1_ANT"], out=local[:],
                                         in0=inc[:], s0=0.0, s1=0.0, imm2=0.0))

        offs_ps = opool.tile([P, 1], dt.float32, tag="offs")
        nc.tensor.matmul(offs_ps[:], lt[:], local[:, FD - 1:FD],
                         start=True, stop=True)
        offs = pool.tile([P, 1], dt.float32)
        chain("v", nc.vector.tensor_copy(offs[:], offs_ps[:]))

        u1 = pool.tile([P, FD], dt.float32, tag="u1")
        chain("v", nc.vector._custom_dve(ops["FRAC_AFF_ANT"], out=u1[:],
                                         in0=local[:], s0=1.0,
                                         s1=offs[:, 0:1], imm2=MAGIC))

        # epilogue operand tiles (filled inside the k-loop slack)
        G = pool.tile([P, FD], dt.float32, tag="G")
        ln_ = pool.tile([P, FD], dt.float32, tag="ln")
        nbf = pool.tile([P, FD], dt.bfloat16, tag="nbf")

        # ---- k-loop ----
        hw = hpool.tile([P, 4, 512], dt.float32, tag="hw")

        def emit_mms(k_, s_ap, start, stop):
            for qo in range(0, FD, 512):
                qn = min(512, FD - qo)
                nc.tensor.matmul(hw[:, qo // 512, 0:qn],
                                 diags[:, k_, :], s_ap[:, qo:qo + qn],
                                 start=start, stop=stop)

        last_group = len(groups) - 1
        for gidx, ks in enumerate(groups):
            n = len(ks)
            if ks == [1]:
                s_t = sqpool.tile([P, 4, FD], dt.bfloat16, tag="sq")
                chain("s", nc.scalar.activation(s_t[:, 0, :], u1[:], AF.Sin,
                                                bias=zero_bias[:, 0:1],
                                                scale=2.0 * PI))
                emit_mms(0, s_t[:, 0, :], True, False)
            else:
                vq = vqpool.tile([P, 4, FD], dt.float32, tag="vq")
                s_t = sqpool.tile([P, 4, FD], dt.bfloat16, tag="sq")
                for j, kk_ in enumerate(ks):
                    chain("v", nc.vector._custom_dve(
                        ops["FRAC4X_ANT"], out=vq[:, j, :], in0=u1[:],
                        s0=float(kk_), s1=0.0, imm2=MAGIC))
                chain("s", nc.scalar.activation(
                    s_t[:, 0:n, :].rearrange("p a f -> p (a f)"),
                    vq[:, 0:n, :].rearrange("p a f -> p (a f)"),
                    AF.Sin, bias=zero_bias[:, 0:1], scale=2.0 * PI))
                for j, kk_ in enumerate(ks):
                    emit_mms(kk_ - 1, s_t[:, j, :], False,
                             gidx == last_group and j == n - 1)
                if gidx == 2:
                    # noise -> bf16 on the ACT engine (scalar copy)
                    chain("s", nc.scalar.copy(nbf[:], noise[:]))

            for ins in inserts.get(gidx, []):
                if ins == "G":
                    chain("v", nc.vector.tensor_tensor(G[:], loud[:], mix[:],
                                                       ALU.mult))
                elif ins == "ln":
                    chain("v", nc.vector.tensor_tensor(ln_[:], loud[:],
                                                       noise[:], ALU.mult))
                else:  # a DVE-poly harmonic
                    dk = ins
                    vd = vdpool.tile([P, FD], dt.float32, tag="vd")
                    chain("v", nc.vector._custom_dve(
                        ops["FRAC4X_ANT"], out=vd[:], in0=u1[:],
                        s0=float(dk), s1=0.0, imm2=MAGIC))
                    sd = sdpool.tile([P, FD], dt.bfloat16, tag="sd")
                    chain("v", nc.vector._custom_dve(
                        ops["SINP7_ANT"], out=sd[:], in0=vd[:],
                        s0=B1, s1=B3, imm2=B5))
                    emit_mms(dk - 1, sd[:], False, False)
                    if dk == dve_ks[-1]:
                        # noise fold-in: hw -= I @ bf16(noise)
                        emit_mms(H, nbf[:], False, False)

        # ---- epilogue: audio = G*hw + ln, then per-row peak normalize ----
        hw_flat = hw[:].rearrange("p q f -> p (q f)")[:, 0:FD]
        e1 = pool.tile([P, FD], dt.float32, tag="scr", bufs=4, name="e1")
        chain("v", nc.vector.tensor_tensor(e1[:], G[:], hw_flat, ALU.mult))
        audio = pool.tile([P, FD], dt.float32, tag="scr", bufs=4, name="audio")
        chain("v", nc.vector.tensor_tensor(audio[:], e1[:], ln_[:], ALU.add))

        pk = pool.tile([P, 1], dt.float32, tag="pk")
        chain("v", nc.vector.tensor_reduce(pk[:], audio[:],
                                           axis=mybir.AxisListType.X,
                                           op=ALU.max,
                                           apply_absolute_value=True))
        pkr = pool.tile([P, 32], dt.float32, tag="pkr")
        chain("v", nc.vector.tensor_copy(pkr[:],
                                         pk[:, 0:1].to_broadcast((P, 32))))
        pkt = pool.tile([P, 32], dt.float32, tag="pkt")
        chain("v", nc.vector.transpose(pkt[:], pkr[:]))
        rowmax = pool.tile([P, 1], dt.float32, tag="rowmax")
        chain("v", nc.vector.tensor_reduce(rowmax[:], pkt[:],
                                           axis=mybir.AxisListType.X,
                                           op=ALU.max))
        pke = pool.tile([P, 1], dt.float32, tag="pke")
        chain("v", nc.vector.tensor_scalar(pke[:], rowmax[:], 1e-6, None,
                                           ALU.add))
        rcp = pool.tile([P, 1], dt.float32, tag="rcp")
        chain("v", nc.vector.reciprocal(rcp[:], pke[:]))
        outt = pool.tile([P, FD], dt.float32, tag="scr", bufs=4, name="outt")
        chain("v", nc.vector.tensor_scalar(outt[:], audio[:], rcp[:, 0:1],
                                           None, ALU.mult))
        nc.sync.dma_start(out_d.ap(), outt[:])

    nc.compile()
    return nc


def kernel(f0, loudness, harmonic_mix, noise):
    if "nc" not in _cache:
        _cache["nc"] = _build()
        _cache["consts"] = _consts()
    nc = _cache["nc"]
    consts = _cache["consts"]

    def shard(a, c):
        return np.ascontiguousarray(
            a[c * RPC:(c + 1) * RPC].astype(f32, copy=False).reshape(P, FD))

    in_maps = []
    for c in range(NCORES):
        in_maps.append({
            "f0": shard(f0, c),
            "loud": shard(loudness, c),
            "mix": shard(harmonic_mix, c),
            "noise": shard(noise, c),
            **consts,
        })

    res = bass_utils.run_bass_kernel_spmd(nc, in_maps, core_ids=list(range(NCORES)))
    outs = [res.results[c]["audio"].reshape(RPC, T) for c in range(NCORES)]
    return np.concatenate(outs, axis=0)


# revision 18
# speedup vs baseline: 1.2782x; 1.1353x over previous
"""DDSP core synthesizer kernel for Trainium2 (8 NeuronCores, data-parallel).

Reference computation (per row of B=32, T=64000):
    f0_hz = 20*exp(f0); phase = cumsum(2*pi*f0_hz/SR)
    hw    = sum_k sin(phase*k)/k   (k = 1..60)
    audio = mix*hw*loud + (1-mix)*noise*loud;  out = audio / (max|audio| + 1e-6)

Device algorithm (phase kept in "turns"), layout [128 partitions = 4 rows x
32 blocks, 2000 free = time-in-block]:
    inc  = exp(f0 + ln(20/SR))                     [ACT Exp]
    u    = single-src cumsum (custom DVE scan); cross-block offsets via a
           triangular matmul [PE]; u1 = frac(u+offs) [custom DVE, 1x]
    per harmonic k (k=1 reuses u1 directly — frac(1*u1) = u1):
        v_k = frac(k*u1)     [custom DVE FRAC4X — hand-authored 2x_2p uop:
                              the 4-slice frac chain duplicated across the
                              8-slice pipe reading both SBUF ports, with
                              perf_max=2 in instruction byte 36 -> runs at
                              2 elem/cycle/lane, ~1.19us per [128,2000] op]
        k <= M_ACT: s_k = sin(2pi v_k) -> bf16     [ACT Sin, one op per
                              GROUP of up to 4 harmonics: same scale for
                              all k, so quads amortize the 352-cyc fixed
                              cost; groups sized 1,2,4.. so ACT starts
                              right after u1]
        k >  M_ACT: s_k = P7(v_k)/|a7| -> bf16     [custom DVE SINP7, deg-7
                              odd minimax poly (max err 2.5e-4), leading
                              coeff normalized so constants are
                              k-independent; |a7|/k goes into the PE diag.
                              Interleaved into the DVE slack while ACT is
                              the pacer.]
        hw += diag(w_k) @ s_k in PSUM              [PE; w_k = 1/k or |a7|/k]
    noise is folded in via PSUM too: hw -= I @ bf16(noise)  [PE]
    epilogue: audio = G*hw + ln with G = loud*mix, ln = loud*noise
              (both computed on the DVE inside its slack windows; GpSimd is
              kept DMA-only — its tensor ops starve the 2x frac of SBUF
              ports); per-row peak normalize (free-reduce + 32x32 transpose).

Sharding: pure data parallel, 4 rows per core, SPMD on cores 0-7.
"""

import sys

sys.path.insert(0, "/opt/trn_rl_repo")

import numpy as np
import ml_dtypes
from contextlib import ExitStack

import concourse.bass as bass
import concourse.tile as tile
from concourse import bacc, mybir
from concourse import bass_utils

f32 = np.float32
dt = mybir.dt

SR = 44100.0
H = 60                      # harmonics
B, T = 32, 64000
NCORES = 8
RPC = B // NCORES           # rows per core = 4
P = 128                     # SBUF partitions
FD = T * RPC // P           # free dim of master tiles = 2000
BPR = P // RPC              # blocks per row = 32
PI = float(np.pi)
MAGIC = float(1.5 * 2.0 ** 23)
EXP_BIAS = float(np.log(20.0 / SR))

# deg-7 odd minimax fit of sin(2*pi*f) on f in [-0.5, 0.5]; max err 2.5e-4
A1, A3, A5, A7 = 6.27863591, -41.09374848, 77.93051701, 56.08683302
B1, B3, B5 = A1 / A7, A3 / A7, A5 / A7     # P = f(B1+y(B3+y(B5-y))) = sin/|a7|

M_ACT = 55                  # harmonics 1..M_ACT on ACT Sin; rest on DVE poly

_cache = {}


def _register_ops():
    """Custom DVE ops. FRAC4X additionally carries a hand-written 2x_2p
    (two-read-port) uop program + perf_max=2 so the engine runs it at
    2 elem/cycle/lane for fp32 SBUF single-source calls."""
    if "ops" in _cache:
        return _cache["ops"]
    from concourse import dve_ops
    from concourse.dve_spec import (Spec, Src0, C0, C1, C2, lower, scan, sq,
                                    AluOp)
    from concourse.dve_spec import _has_src1 as has_src1
    from concourse.dve_uop import (DveOpSpec, UopConfig, InpSel, AluInp,
                                   OutSel, OutPath, Trigger, ENABLE, DelayInp)
    from concourse.dve_uop import AluOp as UAluOp
    from concourse.dve_table_gen import dve_ver_for

    ver = dve_ver_for("TRN2")
    ops = {}

    def base_reg(name, spec, myspec=None):
        if name not in dve_ops._SUB_OPCODE_FOR_NAME:
            row = max(dve_ops._SUB_OPCODE_FOR_NAME.values()) + 1
            assert row < 0x20
            dve_ops._SUB_OPCODE_FOR_NAME[name] = row
        if myspec is None:
            myspec_ = DveOpSpec(
                name=name, opcode=dve_ops.get_dve_sub_opcode(name),
                uops=lower(spec, ver=ver), rd1_en=has_src1(spec))
        else:
            myspec_ = myspec
            myspec_.opcode = dve_ops.get_dve_sub_opcode(name)
            myspec_.validate(ver)

        class _Op(dve_ops.DveOp):
            def compile(self, ver_):
                return myspec_

        op = _Op(name, spec, subdim=False, uops_sha={ver: myspec_.sha(ver)})
        if not any(o.name == name for o in dve_ops.OPS):
            dve_ops.OPS.append(op)
        dve_ops.CUSTOM_DVE_SPECS[name] = spec
        ops[name] = op
        return op

    # ---- FRAC_AFFINE (1x, 5 slices): f = x - rint(x), x = Src0*C0 + C1 ----
    def fa_ref(in0, in1, s0, s1, imm2):
        x = (in0.astype(f32) * f32(s0) + f32(s1)).astype(f32)
        w = ((x + f32(imm2)).astype(f32) - f32(imm2)).astype(f32)
        return (x - w).astype(f32)
    xa = Src0 * C0 + C1
    base_reg("FRAC_AFF_ANT", Spec(body=xa - ((xa + C2) - C2), reference=fa_ref))

    # ---- CUMSUM1 (1x, single-source scan; s0 = per-partition carry-in) ----
    def cs_ref(in0, in1, s0, s1, imm2):
        cs = np.cumsum(in0.astype(f32), axis=-1, dtype=f32)
        s = np.asarray(s0, dtype=f32)
        if s.ndim:
            s = s.reshape(-1, 1)
        return (cs + s).astype(f32)
    base_reg("CUMSUM1_ANT", Spec(body=scan(AluOp.ADD, Src0, init=C0),
                                 reference=cs_ref))

    # ---- SINP7 (1x, 7 slices): out = Src0*(C0 + y*(C1 + y*(C2 - y))) ----
    def sp_ref(in0, in1, s0, s1, imm2):
        f = in0.astype(f32)
        y = (f * f).astype(f32)
        t = (f32(imm2) - y).astype(f32)
        t = (y * t).astype(f32)
        t = (t + f32(s1)).astype(f32)
        t = (y * t).astype(f32)
        t = (t + f32(s0)).astype(f32)
        return (f * t).astype(f32)
    yq = sq(Src0)
    base_reg("SINP7_ANT", Spec(body=Src0 * (C0 + yq * (C1 + yq * (C2 - yq))),
                               reference=sp_ref))

    # ---- FRAC4X (2x_2p): f = x - rint(x), x = Src0*C0 ----
    def f4_ref(in0, in1, s0, s1, imm2):
        x = (in0.astype(f32) * f32(s0)).astype(f32)
        w = ((x + f32(imm2)).astype(f32) - f32(imm2)).astype(f32)
        return (x - w).astype(f32)
    xb = Src0 * C0
    spec4 = Spec(body=xb - ((xb + C2) - C2), reference=f4_ref)
    uops_1x = lower(spec4, ver=ver)

    u = UopConfig()
    u.enable_input(InpSel.SRC_0, 1)    # chain 0: element i  (read port 0)
    u.enable_input(InpSel.CONST_0, 2)  # chain 1: k
    u.enable_input(InpSel.CONST_2, 3)  # chain 2: magic rint constant
    u.enable_input(InpSel.SRC_1, 4)    # chain 3: element i+1 (read port 1)
    u.require_inp0 = ENABLE
    u.require_inp1 = ENABLE
    u.trigger = (Trigger.SRC_TENSOR_DONE, Trigger.NONE, Trigger.NONE)
    b = u.datapath_config
    PD, PA = DelayInp.PREV_DELAY, DelayInp.PREV_ALU_OUT
    # chain A (elem i): blocks 0-3; chain B (elem i+1): blocks 4-7
    b[0].enable_alu(UAluOp.MULTIPLY, AluInp.PREV_DELAY_0, AluInp.PREV_DELAY_1)
    for c in (0, 1, 2, 3):
        b[0].enable_delay_from_src(PD, c)
    b[1].enable_alu(UAluOp.ADD, AluInp.PREV_ALU_OUT, AluInp.PREV_DELAY_2)
    b[1].enable_delay_from_src(PA, 0)
    for c in (1, 2, 3):
        b[1].enable_delay_from_src(PD, c)
    b[2].enable_alu(UAluOp.SUBTRACT, AluInp.PREV_ALU_OUT, AluInp.PREV_DELAY_2)
    for c in (0, 1, 2, 3):
        b[2].enable_delay_from_src(PD, c)
    b[3].enable_alu(UAluOp.SUBTRACT, AluInp.PREV_DELAY_0, AluInp.PREV_ALU_OUT)
    for c in (1, 2, 3):
        b[3].enable_delay_from_src(PD, c)
    b[4].enable_alu(UAluOp.MULTIPLY, AluInp.PREV_DELAY_3, AluInp.PREV_DELAY_1)
    b[4].enable_delay_from_src(PA, 0)
    b[4].enable_delay_from_src(PD, 2)
    b[5].enable_alu(UAluOp.ADD, AluInp.PREV_ALU_OUT, AluInp.PREV_DELAY_2)
    b[5].enable_delay_from_src(PA, 3)
    for c in (0, 2):
        b[5].enable_delay_from_src(PD, c)
    b[6].enable_alu(UAluOp.SUBTRACT, AluInp.PREV_ALU_OUT, AluInp.PREV_DELAY_2)
    for c in (0, 3):
        b[6].enable_delay_from_src(PD, c)
    b[7].enable_alu(UAluOp.SUBTRACT, AluInp.PREV_DELAY_3, AluInp.PREV_ALU_OUT)
    b[7].enable_delay_from_src(PD, 0)
    u.enable_output(OutSel.DELAY_0, OutPath.WR0_LO)   # f(elem i)
    u.enable_output(OutSel.ALU_OUT, OutPath.WR1_LO)   # f(elem i+1)

    spec4x = DveOpSpec(
        name="FRAC4X_ANT", uops=uops_1x, uops_2x=[uops_1x[0]],
        uops_2x_2p=[u], uops_4x=None, perf_max=2, rd1_en=has_src1(spec4))
    base_reg("FRAC4X_ANT", spec4, myspec=spec4x)

    _patch_perf_max({"FRAC4X_ANT": 2})
    _cache["ops"] = ops
    return ops


def _patch_perf_max(pm_ops):
    """_custom_dve builds InstCustomDveAnt without perf_max (byte-36[7:6]) and
    add_instruction stores a copy, so swap the constructor symbol for a
    factory that injects it for our 2x-capable ops."""
    from concourse import bass_isa
    real = mybir.InstCustomDveAnt
    if getattr(bass_isa.InstCustomDveAnt, "_pm_patched", False):
        return

    def make(*args, **kw):
        pm = pm_ops.get(kw.get("op_name"))
        if pm is not None:
            kw.setdefault("perf_max", pm)
        return real(*args, **kw)

    make._pm_patched = True
    bass_isa.InstCustomDveAnt = make


def _consts():
    # lt: exclusive-prefix matmul weights. offs[m] = sum_k lt[k, m] * totals[k]
    kk, mm_ = np.meshgrid(np.arange(P), np.arange(P), indexing="ij")
    lt = ((kk // BPR == mm_ // BPR) & (kk % BPR < mm_ % BPR)).astype(f32)

    # diags[k-1] = diag(w_k): w_k = 1/k (ACT sin) or |a7|/k (normalized DVE
    # poly).  diags[H] = -I for the noise fold-in.  Stored host-side already
    # transposed to [P, H+1, P] so the device DMA is fully contiguous (the
    # strided rearrange DMA clogged the DMA engines for ~23us).
    diags = np.zeros((H + 1, P, P), dtype=np.float64)
    for k in range(1, H + 1):
        w = (1.0 / k) if k <= M_ACT else (A7 / k)
        np.fill_diagonal(diags[k - 1], w)
    np.fill_diagonal(diags[H], -1.0)
    diags = np.ascontiguousarray(
        diags.transpose(1, 0, 2)).astype(ml_dtypes.bfloat16)
    return {"lt": lt, "diags": diags}


def _build():
    ops = _register_ops()
    AF = mybir.ActivationFunctionType
    ALU = mybir.AluOpType

    nc = bacc.Bacc("TRN2", target_bir_lowering=False, debug=False,
                   enable_asserts=True, num_devices=NCORES)

    from bass_rust import add_dep_helper

    f0_d = nc.dram_tensor("f0", [P, FD], dt.float32, kind="ExternalInput")
    loud_d = nc.dram_tensor("loud", [P, FD], dt.float32, kind="ExternalInput")
    mix_d = nc.dram_tensor("mix", [P, FD], dt.float32, kind="ExternalInput")
    noise_d = nc.dram_tensor("noise", [P, FD], dt.float32, kind="ExternalInput")
    lt_d = nc.dram_tensor("lt", [P, P], dt.float32, kind="ExternalInput")
    diags_d = nc.dram_tensor("diags", [P, H + 1, P], dt.bfloat16,
                             kind="ExternalInput")
    out_d = nc.dram_tensor("audio", [P, FD], dt.float32, kind="ExternalOutput")

    # ACT groups (k values per Sin op) then DVE-poly harmonics
    groups = [[1], [2, 3]]
    k = 4
    while k + 3 <= M_ACT:
        groups.append([k, k + 1, k + 2, k + 3])
        k += 4
    if k <= M_ACT:
        groups.append(list(range(k, M_ACT + 1)))
    dve_ks = list(range(M_ACT + 1, H + 1))
    # DVE-stream inserts after group index i. One frac+SINP7 insert costs
    # ~3.4us of DVE but a quad only has ~2.2us of slack under the 6.95us ACT
    # pace, so space the poly harmonics every OTHER quad.
    inserts = {3: ["G"], 4: ["ln"]}
    gi = 5
    for dk in dve_ks:
        inserts.setdefault(gi, []).append(dk)
        gi += 2

    # per-engine in-order chaining: pin the static schedule to emission order
    # (the Tile list-scheduler otherwise hoists ready-early ops, e.g. the
    # epilogue prework, ahead of the phase pipeline)
    prev_i = {}

    def chain(eng, bi):
        p = prev_i.get(eng)
        if p is not None:
            add_dep_helper(bi.ins, p.ins, sync=False, reason="stream-order")
        prev_i[eng] = bi
        return bi

    with tile.TileContext(nc) as tc, ExitStack() as ctx:
        pool = ctx.enter_context(tc.tile_pool(name="sb", bufs=1))
        vqpool = ctx.enter_context(tc.tile_pool(name="vq", bufs=2))
        sqpool = ctx.enter_context(tc.tile_pool(name="sq", bufs=2))
        vdpool = ctx.enter_context(tc.tile_pool(name="vd", bufs=1))
        sdpool = ctx.enter_context(tc.tile_pool(name="sd", bufs=1))
        hpool = ctx.enter_context(tc.tile_pool(name="hps", bufs=1, space="PSUM"))
        opool = ctx.enter_context(tc.tile_pool(name="ops", bufs=1, space="PSUM"))

        exp_bias = pool.tile([P, 1], dt.float32, tag="cbias_exp")
        chain("v", nc.vector.memset(exp_bias[:], EXP_BIAS))
        zero_bias = pool.tile([P, 1], dt.float32, tag="cbias_zero")
        chain("v", nc.vector.memset(zero_bias[:], 0.0))

        # ---- input DMA (GpSimd queues are DMA-only) ----
        # f0 halves land on two queues so the first exp chunk starts sooner
        f0 = pool.tile([P, FD], dt.float32, tag="scr", bufs=4, name="f0")
        HF = FD // 2
        nc.sync.dma_start(f0[:, 0:HF], f0_d.ap()[:, 0:HF])
        nc.scalar.dma_start(f0[:, HF:FD], f0_d.ap()[:, HF:FD])
        lt = pool.tile([P, P], dt.float32)
        nc.gpsimd.dma_start(lt[:], lt_d.ap())
        diags = pool.tile([P, H + 1, P], dt.bfloat16)
        for ko in range(0, H + 1, 16):
            kn = min(16, H + 1 - ko)
            nc.gpsimd.dma_start(diags[:, ko:ko + kn, :],
                                diags_d.ap()[:, ko:ko + kn, :])
        loud = pool.tile([P, FD], dt.float32, tag="loud")
        nc.scalar.dma_start(loud[:], loud_d.ap())
        mix = pool.tile([P, FD], dt.float32, tag="mix")
        nc.scalar.dma_start(mix[:], mix_d.ap())
        noise = pool.tile([P, FD], dt.float32, tag="noise")
        nc.scalar.dma_start(noise[:], noise_d.ap())

        # ---- stage 1: phase accumulation (turns), 2-chunk pipelined ----
        inc = pool.tile([P, FD], dt.float32, tag="scr", bufs=4, name="inc")
        chain("s", nc.scalar.activation(inc[:, 0:HF], f0[:, 0:HF], AF.Exp,
                                        bias=exp_bias[:, 0:1], scale=1.0))
        chain("s", nc.scalar.activation(inc[:, HF:FD], f0[:, HF:FD], AF.Exp,
                                        bias=exp_bias[:, 0:1], scale=1.0))

        local = pool.tile([P, FD], dt.float32, tag="scr", bufs=4, name="local")
        chain("v", nc.vector._custom_dve(ops["CUMSUM1_ANT"],
                                         out=local[:, 0:HF], in0=inc[:, 0:HF],
                                         s0=0.0, s1=0.0, imm2=0.0))
        chain("v", nc.vector._custom_dve(ops["CUMSUM1_ANT"],
                                         out=local[:, HF:FD],
                                         in0=inc[:, HF:FD],
                                         s0=local[:, HF - 1:HF],
                                         s1=0.0, imm2=0.0))

        offs_ps = opool.tile([P, 1], dt.float32, tag="offs")
        nc.tensor.matmul(offs_ps[:], lt[:], local[:, FD - 1:FD],
                         start=True, stop=True)
        offs = pool.tile([P, 1], dt.float32)
        chain("v", nc.vector.tensor_copy(offs[:], offs_ps[:]))

        u1 = pool.tile([P, FD], dt.float32, tag="u1")
        chain("v", nc.vector._custom_dve(ops["FRAC_AFF_ANT"], out=u1[:],
                                         in0=local[:], s0=1.0,
                                         s1=offs[:, 0:1], imm2=MAGIC))

        # epilogue operand tiles (filled inside the k-loop slack)
        G = pool.tile([P, FD], dt.float32, tag="G")
        ln_ = pool.tile([P, FD], dt.float32, tag="ln")
        nbf = pool.tile([P, FD], dt.bfloat16, tag="nbf")

        # ---- k-loop ----
        hw = hpool.tile([P, 4, 512], dt.float32, tag="hw")

        def emit_mms(k_, s_ap, start, stop):
            for qo in range(0, FD, 512):
                qn = min(512, FD - qo)
                nc.tensor.matmul(hw[:, qo // 512, 0:qn],
                                 diags[:, k_, :], s_ap[:, qo:qo + qn],
                                 start=start, stop=stop)

        last_group = len(groups) - 1
        for gidx, ks in enumerate(groups):
            n = len(ks)
            if ks == [1]:
                s_t = sqpool.tile([P, 4, FD], dt.bfloat16, tag="sq")
                chain("s", nc.scalar.activation(s_t[:, 0, :], u1[:], AF.Sin,
                                                bias=zero_bias[:, 0:1],
                                                scale=2.0 * PI))
                emit_mms(0, s_t[:, 0, :], True, False)
            else:
                vq = vqpool.tile([P, 4, FD], dt.float32, tag="vq")
                s_t = sqpool.tile([P, 4, FD], dt.bfloat16, tag="sq")
                for j, kk_ in enumerate(ks):
                    chain("v", nc.vector._custom_dve(
                        ops["FRAC4X_ANT"], out=vq[:, j, :], in0=u1[:],
                        s0=float(kk_), s1=0.0, imm2=MAGIC))
                chain("s", nc.scalar.activation(
                    s_t[:, 0:n, :].rearrange("p a f -> p (a f)"),
                    vq[:, 0:n, :].rearrange("p a f -> p (a f)"),
                    AF.Sin, bias=zero_bias[:, 0:1], scale=2.0 * PI))
                for j, kk_ in enumerate(ks):
                    emit_mms(kk_ - 1, s_t[:, j, :], False,
                             gidx == last_group and j == n - 1)
                if gidx == 2:
                    # noise -> bf16 on the ACT engine (scalar copy)
                    chain("s", nc.scalar.copy(nbf[:], noise[:]))
                if gidx == 6:
                    # noise fold-in: hw -= I @ bf16(noise)
                    emit_mms(H, nbf[:], False, False)

            for ins in inserts.get(gidx, []):
                if ins == "G":
                    chain("v", nc.vector.tensor_tensor(G[:], loud[:], mix[:],
                                                       ALU.mult))
                elif ins == "ln":
                    chain("v", nc.vector.tensor_tensor(ln_[:], loud[:],
                                                       noise[:], ALU.mult))
                else:  # a DVE-poly harmonic
                    dk = ins
                    vd = vdpool.tile([P, FD], dt.float32, tag="vd")
                    chain("v", nc.vector._custom_dve(
                        ops["FRAC4X_ANT"], out=vd[:], in0=u1[:],
                        s0=float(dk), s1=0.0, imm2=MAGIC))
                    sd = sdpool.tile([P, FD], dt.bfloat16, tag="sd")
                    chain("v", nc.vector._custom_dve(
                        ops["SINP7_ANT"], out=sd[:], in0=vd[:],
                        s0=B1, s1=B3, imm2=B5))
                    emit_mms(dk - 1, sd[:], False, False)

        # ---- epilogue: audio = G*hw + ln, then per-row peak normalize.
        # 2-chunk so the first half overlaps the final harmonic's matmuls,
        # and the output DMA is split to overlap the normalize. ----
        hw_flat = hw[:].rearrange("p q f -> p (q f)")[:, 0:FD]
        e1 = pool.tile([P, FD], dt.float32, tag="scr", bufs=4, name="e1")
        audio = pool.tile([P, FD], dt.float32, tag="scr", bufs=4, name="audio")
        pk2 = pool.tile([P, 2], dt.float32, tag="pk2")
        for c in range(2):
            lo, hi = c * HF, (c + 1) * HF
            chain("v", nc.vector.tensor_tensor(e1[:, lo:hi], G[:, lo:hi],
                                               hw_flat[:, lo:hi], ALU.mult))
            chain("v", nc.vector.tensor_tensor(audio[:, lo:hi], e1[:, lo:hi],
                                               ln_[:, lo:hi], ALU.add))
            chain("v", nc.vector.tensor_reduce(pk2[:, c:c + 1],
                                               audio[:, lo:hi],
                                               axis=mybir.AxisListType.X,
                                               op=ALU.max,
                                               apply_absolute_value=True))

        pk = pool.tile([P, 1], dt.float32, tag="pk")
        chain("v", nc.vector.tensor_reduce(pk[:], pk2[:],
                                           axis=mybir.AxisListType.X,
                                           op=ALU.max))
        pkr = pool.tile([P, 32], dt.float32, tag="pkr")
        chain("v", nc.vector.tensor_copy(pkr[:],
                                         pk[:, 0:1].to_broadcast((P, 32))))
        pkt = pool.tile([P, 32], dt.float32, tag="pkt")
        chain("v", nc.vector.transpose(pkt[:], pkr[:]))
        rowmax = pool.tile([P, 1], dt.float32, tag="rowmax")
        chain("v", nc.vector.tensor_reduce(rowmax[:], pkt[:],
                                           axis=mybir.AxisListType.X,
                                           op=ALU.max))
        pke = pool.tile([P, 1], dt.float32, tag="pke")
        chain("v", nc.vector.tensor_scalar(pke[:], rowmax[:], 1e-6, None,
                                           ALU.add))
        rcp = pool.tile([P, 1], dt.float32, tag="rcp")
        chain("v", nc.vector.reciprocal(rcp[:], pke[:]))
        outt = pool.tile([P, FD], dt.float32, tag="scr", bufs=4, name="outt")
        for c in range(2):
            lo, hi = c * HF, (c + 1) * HF
            chain("v", nc.vector.tensor_scalar(outt[:, lo:hi],
                                               audio[:, lo:hi],
                                               rcp[:, 0:1], None, ALU.mult))
            nc.sync.dma_start(out_d.ap()[:, lo:hi], outt[:, lo:hi])

    nc.compile()
    return nc


def kernel(f0, loudness, harmonic_mix, noise):
    if "nc" not in _cache:
        _cache["nc"] = _build()
        _cache["consts"] = _consts()
    nc = _cache["nc"]
    consts = _cache["consts"]

    def shard(a, c):
        return np.ascontiguousarray(
            a[c * RPC:(c + 1) * RPC].astype(f32, copy=False).reshape(P, FD))

    in_maps = []
    for c in range(NCORES):
        in_maps.append({
            "f0": shard(f0, c),
            "loud": shard(loudness, c),
            "mix": shard(harmonic_mix, c),
            "noise": shard(noise, c),
            **consts,
        })

    res = bass_utils.run_bass_kernel_spmd(nc, in_maps, core_ids=list(range(NCORES)))
    outs = [res.results[c]["audio"].reshape(RPC, T) for c in range(NCORES)]
    return np.concatenate(outs, axis=0)
